# revision 3
# baseline (speedup 1.0000x reference)
"""Trainium2 Bass kernel for nn_DNET_61881888800848 (normalized-conv U-Net).

Data-parallel over batch: 8 samples -> 8 NeuronCores, one full network per core.

Scheme: H-folded Toeplitz-band convolution. Feature planes live in DRAM as
[C, Hp, Wp] float32r (pad=2 ring of zeros, "frame" origin at (2,2)). For each
output row-block, a gather DMA builds rhs [(c,hi) partitions, w free]; the
band matrix lhsT [(c,hi), (co,ho)] (built host-side from the conv weights)
contracts channel x kernel-row in one matmul; kernel-x (dx) taps accumulate
in PSUM via shifted rhs column slices. Epilogues compute the normalized-conv
x = nomin/(denom+eps)+b, c = denom/sum(w), xc = x*c on DVE/ACT and write
planes back. Pooling is the exact first-max 2x2 tournament on DVE. Nearest
2x upsampling is materialized (DVE w-broadcast + row-duplicating DMAs).
"""
import sys
sys.path.insert(0, '/opt/trn_rl_repo')
import numpy as np

import concourse.bacc as bacc
import concourse.tile as tile
import concourse.mybir as mybir
from concourse.ap import AP
from concourse.bass_utils import run_bass_kernel_spmd

F32 = mybir.dt.float32
F32R = mybir.dt.float32r
F16 = mybir.dt.float16
U8 = mybir.dt.uint8
ALU = mybir.AluOpType
ACTF = mybir.ActivationFunctionType
EPS = 1e-20

B, H, W = 8, 480, 640
PAD = 2


def dims(h, w):
    return h + 2 * PAD, w + 2 * PAD


# ---------------- host-side weight prep ----------------

def band_lhsT(w, HI, HO, dx):
    """w: [co, ci, kh, kw] -> [(ci,HI), (co,HO)] band for kernel-x tap dx."""
    co_n, ci_n, kh, kw = w.shape
    out = np.zeros((ci_n * HI, co_n * HO), np.float32)
    for co in range(co_n):
        for ho in range(HO):
            for ci in range(ci_n):
                for ky in range(kh):
                    out[ci * HI + ho + ky, co * HO + ho] = w[co, ci, ky, dx]
    return out


def prep_consts(inputs):
    w1, w2, w3 = inputs['w1'], inputs['w2'], inputs['w3']
    w4, w5, w6, w7 = inputs['w4'], inputs['w5'], inputs['w6'], inputs['w7']
    c = {}
    # L1: K=(dx5,hi20)=100, M=(co8,ho16)=128, single matmul (dx folded into K)
    lh1 = np.zeros((100, 128), np.float32)
    for dx in range(5):
        for co in range(8):
            for ho in range(16):
                for ky in range(5):
                    lh1[dx * 20 + ho + ky, co * 16 + ho] = w1[co, 0, ky, dx]
    c['lh1'] = lh1
    # 5x5 8ch: [(c,16),(co,12)] x 5 dx
    c['lh2'] = np.stack([band_lhsT(w2, 16, 12, dx) for dx in range(5)])
    c['lh3'] = np.stack([band_lhsT(w3, 16, 12, dx) for dx in range(5)])
    # 3x3 16ch: [(c16,8),(co8,6)] x 3 dx
    c['lh4'] = np.stack([band_lhsT(w4, 8, 6, dx) for dx in range(3)])
    c['lh5'] = np.stack([band_lhsT(w5, 8, 6, dx) for dx in range(3)])
    c['lh6'] = np.stack([band_lhsT(w6, 8, 6, dx) for dx in range(3)])
    # w7 1x1: [(c8,hi16),(1,ho16)] diag
    lh7 = np.zeros((128, 16), np.float32)
    for ci in range(8):
        for ho in range(16):
            lh7[ci * 16 + ho, ho] = w7[0, ci, 0, 0]
    c['lh7'] = lh7

    def vecs(w_, b_, M_per_co):
        isw = (1.0 / w_.reshape(w_.shape[0], -1).sum(1)).astype(np.float32)
        return (np.repeat(b_.astype(np.float32), M_per_co),
                np.repeat(isw, M_per_co))

    c['bv1'], c['iv1'] = vecs(w1, inputs['b1'], 16)
    c['bv2'], c['iv2'] = vecs(w2, inputs['b2'], 12)
    c['bv3'], c['iv3'] = vecs(w3, inputs['b3'], 12)
    c['bv4'], c['iv4'] = vecs(w4, inputs['b4'], 6)
    c['bv5'], c['iv5'] = vecs(w5, inputs['b5'], 6)
    c['bv6'], c['iv6'] = vecs(w6, inputs['b6'], 6)
    for k in list(c.keys()):
        if k.startswith('lh'):
            c[k] = c[k].astype(np.float16)
    c['b7s'] = float(inputs['b7'][0])
    c['i7s'] = float(1.0 / w7.sum())
    return c


# ---------------- device program ----------------

class Net:
    def __init__(self, nc, tc, pools, P):
        self.nc, self.tc, self.pools, self.P = nc, tc, pools, P

    def plane(self, name, C, Hl, Wl):
        Hp, Wp = dims(Hl, Wl)
        t = self.nc.dram_tensor(name, [C, Hp, Wp], F16)
        return t

    def zero_strips(self, pl, C, Hl, Wl, extra_bottom=0):
        """zero top/bottom pad rows (full width) of a plane."""
        nc, zt = self.nc, self.P['zero']
        Hp, Wp = dims(Hl, Wl)
        n = PAD * Wp
        offs = [0, (Hp - PAD) * Wp]
        if extra_bottom:
            offs.append((Hp - PAD - extra_bottom) * Wp)
        for off in offs:
            nc.gpsimd.dma_start(AP(pl, off, [[Hp * Wp, C], [1, n]]), zt[0:C, 0:n])


def build(nc, con):
    """Build the whole per-core network program."""
    S_in = nc.declare_dram_parameter("S", [H, W], F32, isOutput=False)
    pin = {}
    for k, v in con.items():
        if isinstance(v, np.ndarray):
            dt_ = F16 if k.startswith('lh') else F32
            pin[k] = nc.declare_dram_parameter(k, list(v.shape), dt_, isOutput=False)
    out_x = nc.declare_dram_parameter("out_x", [H, W], F32, isOutput=True)
    out_c = nc.declare_dram_parameter("out_c", [H, W], F32, isOutput=True)

    b7s, i7s = con['b7s'], con['i7s']

    with tile.TileContext(nc) as tc:
        pools = {}
        ctx = []
        import contextlib
        stack = contextlib.ExitStack()
        sb = stack.enter_context(tc.tile_pool(name="sb", bufs=1))
        psp = stack.enter_context(tc.tile_pool(name="ps", bufs=4, space="PSUM"))

        # ---- constants in SBUF
        P = {}
        def load_const(name, shape3):
            ar = con[name]
            if ar.ndim == 3:
                nd, K, M = ar.shape
                t = sb.tile([K, nd * M], F16, tag=name)
                nc.sync.dma_start(t[:], AP(pin[name], 0,
                                           [[M, K], [K * M, nd], [1, M]]))
            else:
                K, M = ar.shape
                t = sb.tile([K, M], F16, tag=name)
                nc.sync.dma_start(t[:], pin[name][:])
            return t
        for nm in ('lh1', 'lh2', 'lh3', 'lh4', 'lh5', 'lh6', 'lh7'):
            P[nm] = load_const(nm, None)
        for nm in ('bv1', 'iv1', 'bv2', 'iv2', 'bv3', 'iv3', 'bv4', 'iv4',
                   'bv5', 'iv5', 'bv6', 'iv6'):
            n = con[nm].shape[0]
            t = sb.tile([n, 1], F32, tag=nm)
            nc.sync.dma_start(t[:], pin[nm][:].unsqueeze(1))
            P[nm] = t
        zt = sb.tile([128, 2 * 644], F16, tag="zero")
        nc.vector.memset(zt[:], 0.0)
        P['zero'] = zt

        net = Net(nc, tc, pools, P)

        # ---- all planes
        pl_sc0 = net.plane("p_sc0", 1, H, W)
        pl_c0 = net.plane("p_c0", 1, H, W)
        pl_xc1, pl_c1 = net.plane("p_xc1", 8, H, W), net.plane("p_c1", 8, H, W)
        pl_xc2, pl_c2 = net.plane("p_xc2", 8, H, W), net.plane("p_c2", 8, H, W)
        pl_x1f = net.plane("p_x1f", 8, H, W)
        pl_c1f = net.plane("p_c1f", 8, H, W)
        pl_xc1f = net.plane("p_xc1f", 8, H, W)
        pl_xc1d, pl_c1d = net.plane("p_xc1d", 8, 240, 320), net.plane("p_c1d", 8, 240, 320)
        pl_xc2h, pl_c2h = net.plane("p_xc2h", 8, 240, 320), net.plane("p_c2h", 8, 240, 320)
        pl_x2d = net.plane("p_x2d", 8, 240, 320)
        pl_c2d = net.plane("p_c2d", 8, 240, 320)
        pl_xc2d = net.plane("p_xc2d", 8, 240, 320)
        pl_xc2dd, pl_c2dd = net.plane("p_xc2dd", 8, 120, 160), net.plane("p_c2dd", 8, 120, 160)
        pl_x3d = net.plane("p_x3d", 8, 120, 160)
        pl_c3d = net.plane("p_c3d", 8, 120, 160)
        pl_xc3d = net.plane("p_xc3d", 8, 120, 160)
        pl_xc4i, pl_c4i = net.plane("p_xc4i", 8, 60, 80), net.plane("p_c4i", 8, 60, 80)
        pl_xc4c, pl_c4c = net.plane("p_xc4c", 8, 60, 80), net.plane("p_c4c", 8, 60, 80)
        pl_xc4f, pl_c4f = net.plane("p_xc4f", 8, 120, 160), net.plane("p_c4f", 8, 120, 160)
        pl_xc34, pl_c34 = net.plane("p_xc34", 8, 120, 160), net.plane("p_c34", 8, 120, 160)
        pl_xc34f, pl_c34f = net.plane("p_xc34f", 8, 240, 320), net.plane("p_c34f", 8, 240, 320)
        pl_xc23, pl_c23 = net.plane("p_xc23", 8, 240, 320), net.plane("p_c23", 8, 240, 320)
        pl_xc23f, pl_c23f = net.plane("p_xc23f", 8, H, W), net.plane("p_c23f", 8, H, W)
        pl_xc6, pl_c6 = net.plane("p_xc6", 8, H, W), net.plane("p_c6", 8, H, W)

        for (pl, C, Hl, Wl) in (
            (pl_sc0, 1, H, W), (pl_c0, 1, H, W),
            (pl_xc1, 8, H, W), (pl_c1, 8, H, W),
            (pl_xc2, 8, H, W), (pl_c2, 8, H, W),
            (pl_x1f, 8, H, W), (pl_c1f, 8, H, W), (pl_xc1f, 8, H, W),
            (pl_xc1d, 8, 240, 320), (pl_c1d, 8, 240, 320),
            (pl_xc2h, 8, 240, 320), (pl_c2h, 8, 240, 320),
            (pl_x2d, 8, 240, 320), (pl_c2d, 8, 240, 320), (pl_xc2d, 8, 240, 320),
            (pl_xc2dd, 8, 120, 160), (pl_c2dd, 8, 120, 160),
            (pl_x3d, 8, 120, 160), (pl_c3d, 8, 120, 160), (pl_xc3d, 8, 120, 160),
            (pl_xc4i, 8, 60, 80), (pl_c4i, 8, 60, 80),
            (pl_xc4c, 8, 60, 80), (pl_c4c, 8, 60, 80),
            (pl_xc4f, 8, 120, 160), (pl_c4f, 8, 120, 160),
            (pl_xc34, 8, 120, 160), (pl_c34, 8, 120, 160),
            (pl_xc34f, 8, 240, 320), (pl_c34f, 8, 240, 320),
            (pl_xc23, 8, 240, 320), (pl_c23, 8, 240, 320),
            (pl_xc23f, 8, H, W), (pl_c23f, 8, H, W),
        ):
            net.zero_strips(pl, C, Hl, Wl)
        net.zero_strips(pl_xc6, 8, H, W, extra_bottom=2)
        net.zero_strips(pl_c6, 8, H, W, extra_bottom=2)

        # ring tiles with zero borders: each (name,width) set is zeroed once at
        # creation; epilogues only ever write the interior cols, so borders
        # stay zero and full-width DMA writes carry the plane's w-pads.
        RB = 2  # ring bufs
        rings = {}
        rctr = [0]
        def next_ring(name, width=648):
            key = (name, width)
            if key not in rings:
                lst = []
                for i in range(RB):
                    t = sb.tile([128, width], F16, tag=f"r_{name}_{width}_{i}")
                    nc.vector.memset(t[:], 0.0)
                    lst.append(t)
                rings[key] = lst
            rctr[0] += 1
            return rings[key][rctr[0] % RB]

        tmp_pool = stack.enter_context(tc.tile_pool(name="tmp", bufs=3))
        rhs_pool = stack.enter_context(tc.tile_pool(name="rhs", bufs=4))

        # ---------------- L0: prep c0 / S*c0 ----------------
        Hp, Wp = dims(H, W)
        for hb in range(4):
            st = tmp_pool.tile([120, W], F32, tag="prep_s")
            nc.sync.dma_start(st[:], AP(S_in, 120 * hb * W, [[W, 120], [1, W]]))
            rc = next_ring('c', 648)
            rs = next_ring('x', 648)
            nc.vector.tensor_scalar(rc[0:120, 2:2 + W], st[:], 0.01, None, ALU.is_gt)
            nc.vector.scalar_tensor_tensor(rs[0:120, 2:2 + W], st[:], 0.01, st[:],
                                           ALU.is_gt, ALU.mult)
            row0 = (2 + 120 * hb) * Wp
            nc.gpsimd.dma_start(AP(pl_c0, row0, [[Wp, 120], [1, 644]]), rc[0:120, 0:644])
            nc.gpsimd.dma_start(AP(pl_sc0, row0, [[Wp, 120], [1, 644]]), rs[0:120, 0:644])

        # ---------------- generic conv epilogue ----------------
        def epilogue(ps_n, ps_d, M, NWD, bvec, ivec, w_x, w_c, w_xc, col0):
            """normalized-conv epilogue. Fast path (no plain-x consumer):
            xc = nomin*isw + b*c exactly (division cancels; eps negligible
            for any nonzero denom, and the denom=0 case gives 0 both ways)."""
            nc.vector.tensor_scalar(w_c[0:M, col0:col0 + NWD], ps_d, ivec[0:M, :],
                                    None, ALU.mult)
            if w_x is None:
                bc = tmp_pool.tile([128, 644], F32, tag="ep_bc")
                nc.vector.tensor_scalar(bc[0:M, 0:NWD], w_c[0:M, col0:col0 + NWD],
                                        bvec[0:M, :], None, ALU.mult)
                nc.vector.scalar_tensor_tensor(w_xc[0:M, col0:col0 + NWD], ps_n,
                                               ivec[0:M, :], bc[0:M, 0:NWD],
                                               ALU.mult, ALU.add)
                return
            de = tmp_pool.tile([128, 644], F32, tag="ep_de")
            nc.scalar.activation(de[0:M, 0:NWD], ps_d, ACTF.Copy, bias=EPS)
            rc = tmp_pool.tile([128, 644], F32, tag="ep_rc")
            nc.vector.reciprocal_approx_fast(rc[0:M, 0:NWD], de[0:M, 0:NWD])
            xt = tmp_pool.tile([128, 644], F32, tag="ep_xt")
            nc.vector.tensor_mul(xt[0:M, 0:NWD], ps_n, rc[0:M, 0:NWD])
            nc.scalar.activation(w_x[0:M, col0:col0 + NWD], xt[0:M, 0:NWD],
                                 ACTF.Identity, bias=bvec[0:M, :])
            if w_xc is not None:
                nc.vector.tensor_mul(w_xc[0:M, col0:col0 + NWD],
                                     w_x[0:M, col0:col0 + NWD],
                                     w_c[0:M, col0:col0 + NWD])

        # ---------------- L1 ----------------
        lh1 = P['lh1']
        for t in range(30):
            rt = rhs_pool.tile([128, 644], F16, tag="rh_n")
            for dx in range(5):
                nc.sync.dma_start(
                    rt[20 * dx:20 * dx + 20, 0:640],
                    AP(pl_sc0, (16 * t) * Wp + dx, [[Wp, 20], [1, 640]]))
            rtc = rhs_pool.tile([128, 644], F16, tag="rh_d")
            for dx in range(5):
                nc.sync.dma_start(
                    rtc[20 * dx:20 * dx + 20, 0:640],
                    AP(pl_c0, (16 * t) * Wp + dx, [[Wp, 20], [1, 640]]))
            rx, rc_, rxc = None, next_ring('c', 648), next_ring('xc', 648)
            for half in range(2):
                ps_n = psp.tile([128, 512], F32, tag="ps_n")
                ps_d = psp.tile([128, 512], F32, tag="ps_d")
                nc.tensor.matmul(ps_n[0:128, 0:320], lh1[0:100, :],
                                 rt[0:100, 320 * half:320 * half + 320],
                                 start=True, stop=True)
                nc.tensor.matmul(ps_d[0:128, 0:320], lh1[0:100, :],
                                 rtc[0:100, 320 * half:320 * half + 320],
                                 start=True, stop=True)
                epilogue(ps_n[0:128, 0:320], ps_d[0:128, 0:320], 128, 320,
                         P['bv1'], P['iv1'], None, rc_, rxc, 2 + 320 * half)
            row0 = (2 + 16 * t) * Wp
            nc.gpsimd.dma_start(
                AP(pl_xc1, row0, [[Hp * Wp, 8], [Wp, 16], [1, 644]]), rxc[:, 0:644])
            nc.gpsimd.dma_start(
                AP(pl_c1, row0, [[Hp * Wp, 8], [Wp, 16], [1, 644]]), rc_[:, 0:644])

        # ---------------- 5x5 8ch conv layer helper ----------------
        def conv5(src_xc, src_c, lh, bvec, ivec, Hl, Wl, dst_x, dst_c, dst_xc):
            Hp_, Wp_ = dims(Hl, Wl)
            NT = Hl // 12
            nhalf = max(1, Wl // 320)
            for t in range(NT):
                rn = rhs_pool.tile([128, 644], F16, tag="rh_n")
                nc.sync.dma_start(
                    rn[:, 0:Wp_],
                    AP(src_xc, 12 * t * Wp_, [[Hp_ * Wp_, 8], [Wp_, 16], [1, Wp_]]))
                rd = rhs_pool.tile([128, 644], F16, tag="rh_d")
                nc.scalar.dma_start(
                    rd[:, 0:Wp_],
                    AP(src_c, 12 * t * Wp_, [[Hp_ * Wp_, 8], [Wp_, 16], [1, Wp_]]))
                rw = 648 if Wl >= 640 else Wp_
                rx = next_ring('x', rw) if dst_x is not None else None
                rc_ = next_ring('c', rw)
                rxc = next_ring('xc', rw) if dst_xc is not None else None
                if Wl >= 320:
                    for half in range(nhalf):
                        ps_n = psp.tile([128, 512], F32, tag="ps_n")
                        ps_d = psp.tile([128, 512], F32, tag="ps_d")
                        for dx in range(5):
                            nc.tensor.matmul(
                                ps_n[0:96, 0:320], lh[:, 96 * dx:96 * dx + 96],
                                rn[:, 320 * half + dx:320 * half + dx + 320],
                                start=(dx == 0), stop=(dx == 4))
                        for dx in range(5):
                            nc.tensor.matmul(
                                ps_d[0:96, 0:320], lh[:, 96 * dx:96 * dx + 96],
                                rd[:, 320 * half + dx:320 * half + dx + 320],
                                start=(dx == 0), stop=(dx == 4))
                        epilogue(ps_n[0:96, 0:320], ps_d[0:96, 0:320], 96, 320,
                                 bvec, ivec, rx, rc_, rxc, 2 + 320 * half)
                else:
                    # batched: this t only (Wl=160: NT=10 w/ t-pairs handled by caller)
                    raise NotImplementedError
                row0 = (2 + 12 * t) * Wp_
                for dst, r in ((dst_x, rx), (dst_c, rc_), (dst_xc, rxc)):
                    if dst is not None:
                        nc.gpsimd.dma_start(
                            AP(dst, row0, [[Hp_ * Wp_, 8], [Wp_, 12], [1, Wp_]]),
                            r[0:96, 0:Wp_])

        # small-W 5x5 conv (Wl in {160, 80}): batch all/2 t-blocks into free dim
        def conv5_small(src_xc, src_c, lh, bvec, ivec, Hl, Wl, dst_x, dst_c, dst_xc, TB):
            Hp_, Wp_ = dims(Hl, Wl)
            NT = Hl // 12
            for g in range(NT // TB):
                rn = rhs_pool.tile([128, 644], F16, tag="rh_n")
                rd = rhs_pool.tile([128, 644], F16, tag="rh_d")
                for tt in range(TB):
                    t = g * TB + tt
                    nc.sync.dma_start(
                        rn[:, Wp_ * tt:Wp_ * tt + Wp_],
                        AP(src_xc, 12 * t * Wp_, [[Hp_ * Wp_, 8], [Wp_, 16], [1, Wp_]]))
                    nc.scalar.dma_start(
                        rd[:, Wp_ * tt:Wp_ * tt + Wp_],
                        AP(src_c, 12 * t * Wp_, [[Hp_ * Wp_, 8], [Wp_, 16], [1, Wp_]]))
                rw = TB * Wp_
                rx = next_ring('x', rw) if dst_x is not None else None
                rc_ = next_ring('c', rw)
                rxc = next_ring('xc', rw) if dst_xc is not None else None
                ps_n = psp.tile([128, 512], F32, tag="ps_n")
                ps_d = psp.tile([128, 512], F32, tag="ps_d")
                NWD = TB * Wl
                rn_ap, rd_ap = rn[:], rd[:]
                for dx in range(5):
                    src_ap = AP(rn_ap.tensor, rn_ap.offset + dx,
                                [list(rn_ap.ap[0]), [Wp_, TB], [1, Wl]])
                    nc.tensor.matmul(ps_n[0:96, 0:NWD], lh[:, 96 * dx:96 * dx + 96],
                                     src_ap, start=(dx == 0), stop=(dx == 4))
                for dx in range(5):
                    src_ap = AP(rd_ap.tensor, rd_ap.offset + dx,
                                [list(rd_ap.ap[0]), [Wp_, TB], [1, Wl]])
                    nc.tensor.matmul(ps_d[0:96, 0:NWD], lh[:, 96 * dx:96 * dx + 96],
                                     src_ap, start=(dx == 0), stop=(dx == 4))
                # epilogue into ring laid out as TB segments of Wp_ each
                for tt in range(TB):
                    epilogue(ps_n[0:96, Wl * tt:Wl * tt + Wl],
                             ps_d[0:96, Wl * tt:Wl * tt + Wl], 96, Wl,
                             bvec, ivec, rx, rc_, rxc, Wp_ * tt + 2)
                for tt in range(TB):
                    t = g * TB + tt
                    row0 = (2 + 12 * t) * Wp_
                    for dst, r in ((dst_x, rx), (dst_c, rc_), (dst_xc, rxc)):
                        if dst is not None:
                            nc.gpsimd.dma_start(
                                AP(dst, row0, [[Hp_ * Wp_, 8], [Wp_, 12], [1, Wp_]]),
                                r[0:96, Wp_ * tt:Wp_ * tt + Wp_])

        # ---------------- pool pass ----------------
        def pool_pass(src_x, src_c, Hl, Wl, dst_xc, dst_c):
            """2x2 first-max pool; writes pooled xc=(x*c/4), c/4 planes."""
            Hp_, Wp_ = dims(Hl, Wl)
            Ho, Wo = Hl // 2, Wl // 2
            Hpo, Wpo = dims(Ho, Wo)
            blocks = []
            h0 = 0
            while h0 < Ho:
                blocks.append(min(h0, Ho - 16))
                h0 += 16
            for hb in set(blocks):
                T = {}
                for (nm, src) in (('x', src_x), ('c', src_c)):
                    for dy in range(2):
                        tt = tmp_pool.tile([128, 644], F16, tag=f"po_{nm}{dy}")
                        nc.sync.dma_start(
                            tt[:, 0:Wl],
                            AP(src, (2 * hb + dy + 2) * Wp_ + 2,
                               [[Hp_ * Wp_, 8], [2 * Wp_, 16], [1, Wl]]))
                        T[(nm, dy)] = tt
                m1 = tmp_pool.tile([128, 324], U8, tag="po_m1")
                m2 = tmp_pool.tile([128, 324], U8, tag="po_m2")
                cw0 = tmp_pool.tile([128, 324], F16, tag="po_cw0")
                cw1 = tmp_pool.tile([128, 324], F16, tag="po_cw1")
                xw0 = tmp_pool.tile([128, 324], F16, tag="po_xw0")
                xw1 = tmp_pool.tile([128, 324], F16, tag="po_xw1")
                for dy, m, cw, xw in ((0, m1, cw0, xw0), (1, m2, cw1, xw1)):
                    ca = T[('c', dy)][:, 0:Wl:2]
                    cb = T[('c', dy)][:, 1:Wl:2]
                    nc.vector.tensor_tensor(m[:, 0:Wo], ca, cb, ALU.is_ge)
                    nc.vector.tensor_tensor(cw[:, 0:Wo], ca, cb, ALU.max)
                    nc.scalar.activation(xw[:, 0:Wo], T[('x', dy)][:, 1:Wl:2], ACTF.Copy)
                    nc.vector.copy_predicated(xw[:, 0:Wo], m[:, 0:Wo],
                                              T[('x', dy)][:, 0:Wl:2])
                m3 = tmp_pool.tile([128, 324], U8, tag="po_m3")
                nc.vector.tensor_tensor(m3[:, 0:Wo], cw0[:, 0:Wo], cw1[:, 0:Wo], ALU.is_ge)
                rc_ = next_ring('c', Wpo)
                nc.vector.tensor_tensor(rc_[:, 2:2 + Wo], cw0[:, 0:Wo], cw1[:, 0:Wo],
                                        ALU.max)
                xds = tmp_pool.tile([128, 324], F16, tag="po_xds")
                nc.scalar.activation(xds[:, 0:Wo], xw1[:, 0:Wo], ACTF.Copy)
                nc.vector.copy_predicated(xds[:, 0:Wo], m3[:, 0:Wo], xw0[:, 0:Wo])
                # c_out = max/4 ; xc_out = x * c_out
                nc.vector.tensor_scalar(rc_[:, 2:2 + Wo], rc_[:, 2:2 + Wo], 0.25,
                                        None, ALU.mult)
                rxc = next_ring('xc', Wpo)
                nc.vector.tensor_mul(rxc[:, 2:2 + Wo], xds[:, 0:Wo], rc_[:, 2:2 + Wo])
                row0 = (2 + hb) * Wpo
                nc.gpsimd.dma_start(
                    AP(dst_c, row0, [[Hpo * Wpo, 8], [Wpo, 16], [1, Wpo]]),
                    rc_[:, 0:Wpo])
                nc.gpsimd.dma_start(
                    AP(dst_xc, row0, [[Hpo * Wpo, 8], [Wpo, 16], [1, Wpo]]),
                    rxc[:, 0:Wpo])

        # ---------------- upsample pass ----------------
        def up_pass(src, dst, Hc, Wc):
            """dst fine plane [2Hc, 2Wc] = nearest-up2 of src coarse [Hc, Wc]."""
            Hpc, Wpc = dims(Hc, Wc)
            Hpf, Wpf = dims(2 * Hc, 2 * Wc)
            blocks = []
            h0 = 0
            while h0 < Hc:
                blocks.append(min(h0, Hc - 16))
                h0 += 16
            for hb in set(blocks):
                ct = tmp_pool.tile([128, 324], F16, tag="up_c")
                nc.sync.dma_start(
                    ct[:, 0:Wc],
                    AP(src, (hb + 2) * Wpc + 2, [[Wpc, 16], [Hpc * Wpc, 8], [1, Wc]]))
                wex = tmp_pool.tile([128, 648], F16, tag="up_w")
                nc.vector.memset(wex[:, 0:2], 0.0)
                nc.vector.memset(wex[:, 2 * Wc + 2:2 * Wc + 4], 0.0)
                ct_ap = ct[:]
                bsrc = AP(ct_ap.tensor, ct_ap.offset, [list(ct_ap.ap[0]), [1, Wc], [0, 2]])
                nc.vector.tensor_copy(wex[:, 2:2 + 2 * Wc], bsrc)
                for dy in range(2):
                    nc.gpsimd.dma_start(
                        AP(dst, (2 * hb + dy + 2) * Wpf,
                           [[2 * Wpf, 16], [Hpf * Wpf, 8], [1, 2 * Wc + 4]]),
                        wex[:, 0:2 * Wc + 4])

        # ---------------- 3x3 16ch conv layer ----------------
        def conv3(srcs_lo, srcs_hi, lh, bvec, ivec, Hl, Wl, dst_x, dst_c, dst_xc,
                  pad0=False, TB=1):
            """srcs_lo: (xc,c) planes for c0-7; srcs_hi: for c8-15. pad0: w6-style."""
            Hp_, Wp_ = dims(Hl, Wl)
            Hout = Hl - 2 if pad0 else Hl
            Wout = Wl - 2 if pad0 else Wl
            NT = (Hout + 5) // 6
            roff = 2 if pad0 else 1
            for g in range(NT // TB):
                rn = rhs_pool.tile([128, 644], F16, tag="rh_n")
                rd = rhs_pool.tile([128, 644], F16, tag="rh_d")
                ts = []
                for tt in range(TB):
                    t = g * TB + tt
                    r0 = 6 * t if 6 * t + 6 <= Hout else Hout - 6
                    ts.append(r0)
                    for (tile_, lo, hi) in ((rn, srcs_lo[0], srcs_hi[0]),
                                            (rd, srcs_lo[1], srcs_hi[1])):
                        nc.sync.dma_start(
                            tile_[0:64, Wp_ * tt:Wp_ * tt + Wp_],
                            AP(lo, (r0 + roff) * Wp_, [[Hp_ * Wp_, 8], [Wp_, 8], [1, Wp_]]))
                        nc.scalar.dma_start(
                            tile_[64:128, Wp_ * tt:Wp_ * tt + Wp_],
                            AP(hi, (r0 + roff) * Wp_, [[Hp_ * Wp_, 8], [Wp_, 8], [1, Wp_]]))
                rw = 645 if pad0 else TB * Wp_
                rx = next_ring('x', rw) if dst_x is not None else None
                rc_ = next_ring('c', rw)
                rxc = next_ring('xc', rw) if dst_xc is not None else None
                # w halves of <=320 output cols
                whs = []
                w0 = 0
                while w0 < Wout:
                    whs.append((w0, min(320, Wout - w0)))
                    w0 += 320
                coff = 2 if pad0 else 1
                for (wo0, wcnt) in whs:
                    ps_n = psp.tile([128, 512], F32, tag="ps_n")
                    ps_d = psp.tile([128, 512], F32, tag="ps_d")
                    for ps, rr in ((ps_n, rn), (ps_d, rd)):
                        rap = rr[:]
                        for dx in range(3):
                            src_ap = AP(rap.tensor, rap.offset + wo0 + dx + coff,
                                        [list(rap.ap[0]), [Wp_, TB], [1, wcnt]])
                            nc.tensor.matmul(ps[0:48, 0:TB * wcnt],
                                             lh[:, 48 * dx:48 * dx + 48],
                                             src_ap, start=(dx == 0), stop=(dx == 2))
                    for tt in range(TB):
                        epilogue(ps_n[0:48, wcnt * tt:wcnt * tt + wcnt],
                                 ps_d[0:48, wcnt * tt:wcnt * tt + wcnt], 48, wcnt,
                                 bvec, ivec, rx, rc_, rxc, Wp_ * tt + 2 + wo0)
                for tt in range(TB):
                    row0 = (2 + ts[tt]) * Wp_
                    for dst, r in ((dst_x, rx), (dst_c, rc_), (dst_xc, rxc)):
                        if dst is not None:
                            nc.gpsimd.dma_start(
                                AP(dst, row0, [[Hp_ * Wp_, 8], [Wp_, 6], [1, Wp_]]),
                                r[0:48, Wp_ * tt:Wp_ * tt + Wp_])

        # ---------------- network ----------------
        conv5(pl_xc1, pl_c1, P['lh2'], P['bv2'], P['iv2'], H, W, None, pl_c2, pl_xc2)
        conv5(pl_xc2, pl_c2, P['lh3'], P['bv3'], P['iv3'], H, W, pl_x1f, pl_c1f, pl_xc1f)
        pool_pass(pl_x1f, pl_c1f, H, W, pl_xc1d, pl_c1d)
        conv5(pl_xc1d, pl_c1d, P['lh2'], P['bv2'], P['iv2'], 240, 320,
              None, pl_c2h, pl_xc2h)
        conv5(pl_xc2h, pl_c2h, P['lh3'], P['bv3'], P['iv3'], 240, 320,
              pl_x2d, pl_c2d, pl_xc2d)
        pool_pass(pl_x2d, pl_c2d, 240, 320, pl_xc2dd, pl_c2dd)
        conv5_small(pl_xc2dd, pl_c2dd, P['lh2'], P['bv2'], P['iv2'], 120, 160,
                    pl_x3d, pl_c3d, pl_xc3d, TB=2)
        pool_pass(pl_x3d, pl_c3d, 120, 160, pl_xc4i, pl_c4i)
        conv5_small(pl_xc4i, pl_c4i, P['lh2'], P['bv2'], P['iv2'], 60, 80,
                    None, pl_c4c, pl_xc4c, TB=5)
        up_pass(pl_xc4c, pl_xc4f, 60, 80)
        up_pass(pl_c4c, pl_c4f, 60, 80)
        conv3((pl_xc3d, pl_c3d), (pl_xc4f, pl_c4f), P['lh4'], P['bv4'], P['iv4'],
              120, 160, None, pl_c34, pl_xc34, TB=2)
        up_pass(pl_xc34, pl_xc34f, 120, 160)
        up_pass(pl_c34, pl_c34f, 120, 160)
        conv3((pl_xc2d, pl_c2d), (pl_xc34f, pl_c34f), P['lh5'], P['bv5'], P['iv5'],
              240, 320, None, pl_c23, pl_xc23, TB=1)
        up_pass(pl_xc23, pl_xc23f, 240, 320)
        up_pass(pl_c23, pl_c23f, 240, 320)
        conv3((pl_xc23f, pl_c23f), (pl_xc1f, pl_c1f), P['lh6'], P['bv6'], P['iv6'],
              H, W, None, pl_c6, pl_xc6, pad0=True, TB=1)

        # ---------------- L11: w7 1x1 ----------------
        lh7 = P['lh7']
        for t in range(30):
            rn = rhs_pool.tile([128, 644], F16, tag="rh_n")
            nc.sync.dma_start(
                rn[:, 0:640],
                AP(pl_xc6, (16 * t + 1) * Wp + 1, [[Hp * Wp, 8], [Wp, 16], [1, 640]]))
            rd = rhs_pool.tile([128, 644], F16, tag="rh_d")
            nc.scalar.dma_start(
                rd[:, 0:640],
                AP(pl_c6, (16 * t + 1) * Wp + 1, [[Hp * Wp, 8], [Wp, 16], [1, 640]]))
            for half in range(2):
                ps_n = psp.tile([128, 512], F32, tag="ps_n")
                ps_d = psp.tile([128, 512], F32, tag="ps_d")
                nc.tensor.matmul(ps_n[0:16, 0:320], lh7[:],
                                 rn[:, 320 * half:320 * half + 320],
                                 start=True, stop=True)
                nc.tensor.matmul(ps_d[0:16, 0:320], lh7[:],
                                 rd[:, 320 * half:320 * half + 320],
                                 start=True, stop=True)
                de = tmp_pool.tile([128, 644], F32, tag="ep_de")
                nc.scalar.activation(de[0:16, 0:320], ps_d[0:16, 0:320], ACTF.Copy,
                                     bias=EPS)
                rcp = tmp_pool.tile([128, 644], F32, tag="ep_rc")
                nc.vector.reciprocal_approx_fast(rcp[0:16, 0:320], de[0:16, 0:320])
                xt = tmp_pool.tile([128, 644], F32, tag="ep_xt")
                nc.vector.tensor_mul(xt[0:16, 0:320], ps_n[0:16, 0:320],
                                     rcp[0:16, 0:320])
                xo = tmp_pool.tile([128, 644], F32, tag="f_xo")
                nc.vector.tensor_scalar(xo[0:16, 0:320], xt[0:16, 0:320], b7s,
                                        None, ALU.add)
                co_ = tmp_pool.tile([128, 644], F32, tag="f_co")
                nc.vector.tensor_scalar(co_[0:16, 0:320], ps_d[0:16, 0:320], i7s,
                                        None, ALU.mult)
                nc.gpsimd.dma_start(
                    AP(out_x, (16 * t) * W + 320 * half, [[W, 16], [1, 320]]),
                    xo[0:16, 0:320])
                nc.gpsimd.dma_start(
                    AP(out_c, (16 * t) * W + 320 * half, [[W, 16], [1, 320]]),
                    co_[0:16, 0:320])

        stack.close()
    nc.finalize()
    return nc


_CACHE = {}
TRACE = False
LAST = None


def kernel(**inputs):
    import time as _t
    key = 0
    if key not in _CACHE:
        _t0 = _t.time()
        con = prep_consts(inputs)
        print(f"[kernel] consts done {_t.time()-_t0:.1f}s", flush=True)
        nc = bacc.Bacc("TRN2", target_bir_lowering=False, debug=False)
        build(nc, con)
        print(f"[kernel] build+finalize done {_t.time()-_t0:.1f}s", flush=True)
        _CACHE[key] = (nc, con)
    nc, con = _CACHE[key]

    S = np.asarray(inputs['S'], np.float32)  # [8,1,480,640]
    in_maps = []
    for b in range(B):
        m = {'S': np.ascontiguousarray(S[b, 0])}
        for k, v in con.items():
            if isinstance(v, np.ndarray):
                m[k] = v
        in_maps.append(m)
    print("[kernel] launching run_bass_kernel_spmd", flush=True)
    r = run_bass_kernel_spmd(nc, in_maps, list(range(B)), trace=TRACE)
    global LAST
    LAST = r
    res = r.results
    if TRACE and r.exec_time_ns:
        print(f"HW exec time: {r.exec_time_ns} ns", flush=True)
    print("[kernel] run done", flush=True)
    xout = np.stack([res[b]['out_x'] for b in range(B)])[:, None]
    cout = np.stack([res[b]['out_c'] for b in range(B)])[:, None]
    return xout, cout



# revision 14
# speedup vs baseline: 1.2956x; 1.2956x over previous
"""Trainium2 Bass kernel for nn_DNET_61881888800848 (normalized-conv U-Net).

Data-parallel over batch: 8 samples -> 8 NeuronCores, one full network per core.

v2 scheme: H-folded Toeplitz-band convolution with row-interleaved (xc|c)
plane pairs. Feature pairs live in DRAM as [C, Hp, 2, Wp] f16 -- each frame
row stores [xc row (Wp) | conf row (Wp)] contiguously, pad=2 zero ring, frame
origin (2,2). This keeps every conv rhs gather and plane write a single <=3-dim
DMA: rhs tiles are [(ci,hi) partitions, (xc|c) x Wp cols], the band matmuls
accumulate kernel-x taps in PSUM for nomin (xc seg) and denom (c seg), and the
epilogue is 2 ACT ops + 1 DVE op (xc = isw*nomin + (b*isw)*denom, which equals
(nomin/denom + b) * c exactly; c = isw*denom), writing one interleaved ring ->
one write DMA per block. Pooling gathers (xc, c) at conf-argmax and scales by
1/4 (xc_pool = xc_sel/4 identically -- no division, no x planes anywhere).
Upsampling is DVE w-broadcast + row-duplicated interleaved writes.
"""
import sys
sys.path.insert(0, '/opt/trn_rl_repo')
import numpy as np

import concourse.bacc as bacc
import concourse.tile as tile
import concourse.mybir as mybir
from concourse.ap import AP
from concourse.bass_utils import run_bass_kernel_spmd

F32 = mybir.dt.float32
F16 = mybir.dt.float16
U8 = mybir.dt.uint8
ALU = mybir.AluOpType
ACTF = mybir.ActivationFunctionType
EPS = 1e-20

B, H, W = 8, 480, 640
PAD = 2
RHS_W = 1344  # fixed rhs tile width


def dims(h, w):
    return h + 2 * PAD, w + 2 * PAD


# ---------------- host-side weight prep ----------------

def band_lhsT(w, HI, HO, dx):
    """w: [co, ci, kh, kw] -> [(ci,HI), (co,HO)] band for kernel-x tap dx."""
    co_n, ci_n, kh, kw = w.shape
    out = np.zeros((ci_n * HI, co_n * HO), np.float32)
    for co in range(co_n):
        for ho in range(HO):
            for ci in range(ci_n):
                for ky in range(kh):
                    out[ci * HI + ho + ky, co * HO + ho] = w[co, ci, ky, dx]
    return out


def prep_consts(inputs):
    w1, w2, w3 = inputs['w1'], inputs['w2'], inputs['w3']
    w4, w5, w6, w7 = inputs['w4'], inputs['w5'], inputs['w6'], inputs['w7']
    c = {}
    # L1: K=(dx5,hi20)=100, M=(co8,ho16)=128, single matmul (dx folded into K)
    lh1 = np.zeros((100, 128), np.float32)
    for dx in range(5):
        for co in range(8):
            for ho in range(16):
                for ky in range(5):
                    lh1[dx * 20 + ho + ky, co * 16 + ho] = w1[co, 0, ky, dx]
    c['lh1'] = lh1
    # 5x5 8ch: [(c,16),(co,12)] x 5 dx
    c['lh2'] = np.stack([band_lhsT(w2, 16, 12, dx) for dx in range(5)])
    c['lh3'] = np.stack([band_lhsT(w3, 16, 12, dx) for dx in range(5)])
    # 3x3 16ch: [(c16,8),(co8,6)] x 3 dx
    c['lh4'] = np.stack([band_lhsT(w4, 8, 6, dx) for dx in range(3)])
    c['lh5'] = np.stack([band_lhsT(w5, 8, 6, dx) for dx in range(3)])
    c['lh6'] = np.stack([band_lhsT(w6, 8, 6, dx) for dx in range(3)])
    # w7 1x1: [(c8,hi16),(1,ho16)] diag
    lh7 = np.zeros((128, 16), np.float32)
    for ci in range(8):
        for ho in range(16):
            lh7[ci * 16 + ho, ho] = w7[0, ci, 0, 0]
    c['lh7'] = lh7

    def vecs(w_, b_, M_per_co):
        isw = (1.0 / w_.reshape(w_.shape[0], -1).sum(1)).astype(np.float32)
        bw = (np.asarray(b_, np.float32) * isw).astype(np.float32)
        return (np.repeat(isw, M_per_co), np.repeat(bw, M_per_co))

    c['iv1'], c['bi1'] = vecs(w1, inputs['b1'], 16)
    c['iv2'], c['bi2'] = vecs(w2, inputs['b2'], 12)
    c['iv3'], c['bi3'] = vecs(w3, inputs['b3'], 12)
    c['iv4'], c['bi4'] = vecs(w4, inputs['b4'], 6)
    c['iv5'], c['bi5'] = vecs(w5, inputs['b5'], 6)
    c['iv6'], c['bi6'] = vecs(w6, inputs['b6'], 6)
    for k in list(c.keys()):
        if k.startswith('lh'):
            c[k] = c[k].astype(np.float16)
    c['b7s'] = float(inputs['b7'][0])
    c['i7s'] = float(1.0 / w7.sum())
    return c


def fap(t, col_off, free_dims, p0=0, pn=None):
    """AP into SBUF tile t: partitions [p0:pn), explicit free dims at col_off."""
    a = t[:] if pn is None else t[p0:pn]
    return AP(a.tensor, a.offset + col_off, [list(a.ap[0])] + free_dims)


# ---------------- device program ----------------

DEBUG_DUMPS = False


def build(nc, con):
    S_in = nc.declare_dram_parameter("S", [H, W], F32, isOutput=False)
    pin = {}
    for k, v in con.items():
        if isinstance(v, np.ndarray):
            dt_ = F16 if k.startswith('lh') else F32
            pin[k] = nc.declare_dram_parameter(k, list(v.shape), dt_, isOutput=False)
    out_x = nc.declare_dram_parameter("out_x", [H, W], F32, isOutput=True)
    out_c = nc.declare_dram_parameter("out_c", [H, W], F32, isOutput=True)

    b7s, i7s = con['b7s'], con['i7s']
    Hp, Wp = dims(H, W)
    W2 = 2 * Wp  # full-res interleaved row pitch

    with tile.TileContext(nc) as tc:
        import contextlib
        stack = contextlib.ExitStack()
        sb = stack.enter_context(tc.tile_pool(name="sb", bufs=1))
        psp = stack.enter_context(tc.tile_pool(name="ps", bufs=4, space="PSUM"))
        tmp_pool = stack.enter_context(tc.tile_pool(name="tmp", bufs=4))
        rhs_pool = stack.enter_context(tc.tile_pool(name="rhs", bufs=6))

        # ---- constants in SBUF
        P = {}
        def load_const(name):
            ar = con[name]
            if ar.ndim == 3:
                nd, K, M = ar.shape
                t = sb.tile([K, nd * M], F16, tag=name)
                nc.sync.dma_start(t[:], AP(pin[name], 0,
                                           [[M, K], [K * M, nd], [1, M]]))
            else:
                K, M = ar.shape
                t = sb.tile([K, M], F16, tag=name)
                nc.sync.dma_start(t[:], pin[name][:])
            return t
        for nm in ('lh1', 'lh2', 'lh3', 'lh4', 'lh5', 'lh6', 'lh7'):
            P[nm] = load_const(nm)
        for nm in ('iv1', 'bi1', 'iv2', 'bi2', 'iv3', 'bi3', 'iv4', 'bi4',
                   'iv5', 'bi5', 'iv6', 'bi6'):
            n = con[nm].shape[0]
            t = sb.tile([n, 1], F32, tag=nm)
            nc.sync.dma_start(t[:], pin[nm][:].unsqueeze(1))
            P[nm] = t
        zt = sb.tile([128, 2576], F16, tag="zero")
        nc.vector.memset(zt[:], 0.0)

        # ---- plane pairs: [C, Hp, 2, Wp] f16 (each row: [xc | conf])
        def plane(name, C, Hl, Wl):
            Hp_, Wp_ = dims(Hl, Wl)
            return nc.dram_tensor(name, [C, Hp_, 2 * Wp_], F16)

        def zero_strips(pl, C, Hl, Wl, extra_bottom=0):
            Hp_, Wp_ = dims(Hl, Wl)
            CS = Hp_ * 2 * Wp_
            n = PAD * 2 * Wp_
            offs = [0, (Hp_ - PAD) * 2 * Wp_]
            if extra_bottom:
                offs.append((Hp_ - PAD - extra_bottom) * 2 * Wp_)
            for off in offs:
                nc.gpsimd.dma_start(
                    AP(pl, off, [[CS, C], [1, n]]), zt[0:C, 0:n])

        # P0 gets one spare row: L1's dx-shifted full-width gather reads a few
        # elements past the last frame row.
        P0 = nc.dram_tensor("P0", [1, (Hp + 1) * 2 * Wp], F16)
        P1 = plane("P1", 8, H, W)
        P2 = plane("P2", 8, H, W)
        P1f = plane("P1f", 8, H, W)
        P1d = plane("P1d", 8, 240, 320)
        P2h = plane("P2h", 8, 240, 320)
        P2d = plane("P2d", 8, 240, 320)
        P2dd = plane("P2dd", 8, 120, 160)
        P3d = plane("P3d", 8, 120, 160)
        P4i = plane("P4i", 8, 60, 80)
        P4c = plane("P4c", 8, 60, 80)
        P4f = plane("P4f", 8, 120, 160)
        P34 = plane("P34", 8, 120, 160)
        P34f = plane("P34f", 8, 240, 320)
        P23 = plane("P23", 8, 240, 320)
        P23f = plane("P23f", 8, H, W)
        P6 = plane("P6", 8, H, W)

        for (pl, C, Hl, Wl) in (
            (P0, 1, H, W), (P1, 8, H, W), (P2, 8, H, W), (P1f, 8, H, W),
            (P1d, 8, 240, 320), (P2h, 8, 240, 320), (P2d, 8, 240, 320),
            (P2dd, 8, 120, 160), (P3d, 8, 120, 160),
            (P4i, 8, 60, 80), (P4c, 8, 60, 80),
            (P4f, 8, 120, 160), (P34, 8, 120, 160),
            (P34f, 8, 240, 320), (P23, 8, 240, 320), (P23f, 8, H, W),
        ):
            zero_strips(pl, C, Hl, Wl)
        zero_strips(P6, 8, H, W, extra_bottom=2)

        # ring tiles with zero borders: memset once at creation; epilogues
        # write only interior cols of each segment, so full-width DMA writes
        # carry the plane's w-pads.
        RB = 3
        rings = {}
        rctr = [0]
        def next_ring(name, width):
            key = (name, width)
            if key not in rings:
                lst = []
                for i in range(RB):
                    t = sb.tile([128, width], F16, tag=f"r_{name}_{width}_{i}")
                    nc.vector.memset(t[:], 0.0)
                    lst.append(t)
                rings[key] = lst
            rctr[0] += 1
            return rings[key][rctr[0] % RB]

        # ---------------- generic conv epilogue ----------------
        # xc = isw*nomin + (b*isw)*denom  (== (nomin/denom + b) * c exactly)
        # c  = isw*denom
        def epilogue(ps_n, ps_d, M, NWD, ivec, bivec, ring, xc_col, c_col):
            nc.scalar.activation(ring[0:M, c_col:c_col + NWD], ps_d,
                                 ACTF.Copy, scale=ivec[0:M, :])
            t2 = tmp_pool.tile([128, 512], F32, tag="ep_t2")
            nc.scalar.activation(t2[0:M, 0:NWD], ps_d, ACTF.Copy,
                                 scale=bivec[0:M, :])
            nc.vector.scalar_tensor_tensor(ring[0:M, xc_col:xc_col + NWD],
                                           ps_n, ivec[0:M, :], t2[0:M, 0:NWD],
                                           ALU.mult, ALU.add)

        # ---------------- L0: prep (S*c0 | c0) ----------------
        for hb in range(4):
            st = tmp_pool.tile([120, W], F32, tag="prep_s")
            nc.sync.dma_start(st[:], AP(S_in, 120 * hb * W, [[W, 120], [1, W]]))
            r = next_ring('out', 1288)
            nc.vector.tensor_scalar(r[0:120, 644 + 2:644 + 2 + W], st[:], 0.01,
                                    None, ALU.is_gt)
            nc.vector.scalar_tensor_tensor(r[0:120, 2:2 + W], st[:], 0.01,
                                           st[:], ALU.is_gt, ALU.mult)
            nc.gpsimd.dma_start(
                AP(P0, (2 + 120 * hb) * W2, [[W2, 120], [1, 1288]]),
                r[0:120, 0:1288])

        # ---------------- L1 ----------------
        lh1 = P['lh1']
        for t in range(30):
            rt = rhs_pool.tile([128, RHS_W], F16, tag="rh_n")
            # partitions (dx5, hi20); 1288 contiguous = (xc 644 | c 644), the
            # dx column shift applies consistently to both segments
            nc.sync.dma_start(
                rt[0:100, 0:1288],
                AP(P0, (16 * t) * W2, [[1, 5], [W2, 20], [1, 1288]]))
            r = next_ring('out', 1288)
            for half in range(2):
                ps_n = psp.tile([128, 512], F32, tag="ps_n")
                ps_d = psp.tile([128, 512], F32, tag="ps_d")
                nc.tensor.matmul(ps_n[0:128, 0:320], lh1[0:100, :],
                                 rt[0:100, 320 * half:320 * half + 320],
                                 start=True, stop=True)
                nc.tensor.matmul(ps_d[0:128, 0:320], lh1[0:100, :],
                                 rt[0:100, 644 + 320 * half:644 + 320 * half + 320],
                                 start=True, stop=True)
                epilogue(ps_n[0:128, 0:320], ps_d[0:128, 0:320], 128, 320,
                         P['iv1'], P['bi1'], r, 2 + 320 * half, 644 + 2 + 320 * half)
            nc.gpsimd.dma_start(
                AP(P1, (2 + 16 * t) * W2, [[2 * Hp * Wp, 8], [1, 16 * 1288]]),
                r[:, 0:1288])

        # ---------------- 5x5 8ch conv layer (W >= 320) ----------------
        def conv5(src, lh, iv, bi, Hl, Wl, dst):
            Hp_, Wp_ = dims(Hl, Wl)
            CS = Hp_ * 2 * Wp_
            NT = Hl // 12
            nhalf = Wl // 320
            for t in range(NT):
                rn = rhs_pool.tile([128, RHS_W], F16, tag="rh_n")
                nc.sync.dma_start(
                    rn[:, 0:2 * Wp_],
                    AP(src, 12 * t * 2 * Wp_,
                       [[CS, 8], [2 * Wp_, 16], [1, 2 * Wp_]]))
                r = next_ring('out', 2 * Wp_)
                for half in range(nhalf):
                    ps_n = psp.tile([128, 512], F32, tag="ps_n")
                    ps_d = psp.tile([128, 512], F32, tag="ps_d")
                    for dx in range(5):
                        nc.tensor.matmul(
                            ps_n[0:96, 0:320], lh[:, 96 * dx:96 * dx + 96],
                            rn[:, 320 * half + dx:320 * half + dx + 320],
                            start=(dx == 0), stop=(dx == 4))
                    for dx in range(5):
                        nc.tensor.matmul(
                            ps_d[0:96, 0:320], lh[:, 96 * dx:96 * dx + 96],
                            rn[:, Wp_ + 320 * half + dx:Wp_ + 320 * half + dx + 320],
                            start=(dx == 0), stop=(dx == 4))
                    epilogue(ps_n[0:96, 0:320], ps_d[0:96, 0:320], 96, 320,
                             iv, bi, r, 2 + 320 * half, Wp_ + 2 + 320 * half)
                nc.gpsimd.dma_start(
                    AP(dst, (2 + 12 * t) * 2 * Wp_,
                       [[CS, 8], [1, 12 * 2 * Wp_]]),
                    r[0:96, 0:2 * Wp_])

        # small-W 5x5 conv (Wl in {160, 80}): batch TB t-blocks into free dim
        def conv5_small(src, lh, iv, bi, Hl, Wl, dst, TB):
            Hp_, Wp_ = dims(Hl, Wl)
            CS = Hp_ * 2 * Wp_
            NT = Hl // 12
            for g in range(NT // TB):
                rn = rhs_pool.tile([128, RHS_W], F16, tag="rh_n")
                for tt in range(TB):
                    t = g * TB + tt
                    nc.sync.dma_start(
                        rn[:, tt * 2 * Wp_:(tt + 1) * 2 * Wp_],
                        AP(src, 12 * t * 2 * Wp_,
                           [[CS, 8], [2 * Wp_, 16], [1, 2 * Wp_]]))
                r = next_ring('out', TB * 2 * Wp_)
                ps_n = psp.tile([128, 512], F32, tag="ps_n")
                ps_d = psp.tile([128, 512], F32, tag="ps_d")
                NWD = TB * Wl
                for dx in range(5):
                    nc.tensor.matmul(
                        ps_n[0:96, 0:NWD], lh[:, 96 * dx:96 * dx + 96],
                        fap(rn, dx, [[2 * Wp_, TB], [1, Wl]]),
                        start=(dx == 0), stop=(dx == 4))
                for dx in range(5):
                    nc.tensor.matmul(
                        ps_d[0:96, 0:NWD], lh[:, 96 * dx:96 * dx + 96],
                        fap(rn, Wp_ + dx, [[2 * Wp_, TB], [1, Wl]]),
                        start=(dx == 0), stop=(dx == 4))
                for tt in range(TB):
                    epilogue(ps_n[0:96, Wl * tt:Wl * tt + Wl],
                             ps_d[0:96, Wl * tt:Wl * tt + Wl], 96, Wl,
                             iv, bi, r, 2 * Wp_ * tt + 2, 2 * Wp_ * tt + Wp_ + 2)
                for tt in range(TB):
                    t = g * TB + tt
                    nc.gpsimd.dma_start(
                        AP(dst, (2 + 12 * t) * 2 * Wp_,
                           [[CS, 8], [1, 12 * 2 * Wp_]]),
                        r[0:96, tt * 2 * Wp_:(tt + 1) * 2 * Wp_])

        # ---------------- pool pass: 2x2 first-max on conf ----------------
        # xc_out = xc[argmax]/4, c_out = c[argmax]/4 (no division needed)
        def pool_pass(src, Hl, Wl, dst):
            Hp_, Wp_ = dims(Hl, Wl)
            CS = Hp_ * 2 * Wp_
            Ho, Wo = Hl // 2, Wl // 2
            Hpo, Wpo = dims(Ho, Wo)
            CSo = Hpo * 2 * Wpo
            blocks = []
            h0 = 0
            while h0 < Ho:
                blocks.append(min(h0, Ho - 16))
                h0 += 16
            for hb in sorted(set(blocks)):
                T = {}
                for (nm, soff) in (('x', 0), ('c', Wp_)):
                    for dy in range(2):
                        tt = tmp_pool.tile([128, 640], F16, tag=f"po_{nm}{dy}")
                        nc.scalar.dma_start(
                            tt[:, 0:Wl],
                            AP(src, (2 * hb + dy + 2) * 2 * Wp_ + soff + 2,
                               [[CS, 8], [4 * Wp_, 16], [1, Wl]]))
                        T[(nm, dy)] = tt
                cw = {}
                xw = {}
                for dy in range(2):
                    m = tmp_pool.tile([128, 324], U8, tag=f"po_m{dy}")
                    cwt = tmp_pool.tile([128, 324], F16, tag=f"po_cw{dy}")
                    xwt = tmp_pool.tile([128, 324], F16, tag=f"po_xw{dy}")
                    ca = T[('c', dy)][:, 0:Wl:2]
                    cb = T[('c', dy)][:, 1:Wl:2]
                    nc.vector.tensor_tensor(m[:, 0:Wo], ca, cb, ALU.is_ge)
                    nc.vector.tensor_tensor(cwt[:, 0:Wo], ca, cb, ALU.max)
                    nc.scalar.activation(xwt[:, 0:Wo], T[('x', dy)][:, 1:Wl:2],
                                         ACTF.Copy)
                    nc.vector.copy_predicated(xwt[:, 0:Wo], m[:, 0:Wo],
                                              T[('x', dy)][:, 0:Wl:2])
                    cw[dy] = cwt
                    xw[dy] = xwt
                m3 = tmp_pool.tile([128, 324], U8, tag="po_m3")
                nc.vector.tensor_tensor(m3[:, 0:Wo], cw[0][:, 0:Wo],
                                        cw[1][:, 0:Wo], ALU.is_ge)
                r = next_ring('po', 2 * Wpo)
                nc.vector.tensor_tensor(r[:, Wpo + 2:Wpo + 2 + Wo],
                                        cw[0][:, 0:Wo], cw[1][:, 0:Wo], ALU.max)
                nc.vector.tensor_scalar(r[:, Wpo + 2:Wpo + 2 + Wo],
                                        r[:, Wpo + 2:Wpo + 2 + Wo], 0.25,
                                        None, ALU.mult)
                nc.scalar.activation(r[:, 2:2 + Wo], xw[1][:, 0:Wo], ACTF.Copy)
                nc.vector.copy_predicated(r[:, 2:2 + Wo], m3[:, 0:Wo],
                                          xw[0][:, 0:Wo])
                nc.vector.tensor_scalar(r[:, 2:2 + Wo], r[:, 2:2 + Wo], 0.25,
                                        None, ALU.mult)
                nc.gpsimd.dma_start(
                    AP(dst, (2 + hb) * 2 * Wpo, [[CSo, 8], [1, 16 * 2 * Wpo]]),
                    r[:, 0:2 * Wpo])

        # ---------------- upsample pass (both planes of a pair) ----------------
        def up_pass(src, dst, Hc, Wc):
            Hpc, Wpc = dims(Hc, Wc)
            Hpf, Wpf = dims(2 * Hc, 2 * Wc)
            CSc = Hpc * 2 * Wpc
            CSf = Hpf * 2 * Wpf
            blocks = []
            h0 = 0
            while h0 < Hc:
                blocks.append(min(h0, Hc - 16))
                h0 += 16
            for hb in sorted(set(blocks)):
                ct = tmp_pool.tile([128, 648], F16, tag="up_c")
                nc.scalar.dma_start(
                    ct[:, 0:2 * Wpc],
                    AP(src, (hb + 2) * 2 * Wpc,
                       [[2 * Wpc, 16], [CSc, 8], [1, 2 * Wpc]]))
                wex = next_ring('up', 2 * Wpf)
                # width-double both segments: out[seg][2w+e] = in[seg][w]
                nc.vector.tensor_copy(
                    fap(wex, 2, [[Wpf, 2], [1, 2 * Wc]]),
                    fap(ct, 2, [[Wpc, 2], [1, Wc], [0, 2]]))
                for dy in range(2):
                    nc.gpsimd.dma_start(
                        AP(dst, (2 * hb + dy + 2) * 2 * Wpf,
                           [[4 * Wpf, 16], [CSf, 8], [1, 2 * Wpf]]),
                        wex[:, 0:2 * Wpf])

        # ---------------- 3x3 16ch conv layer ----------------
        def conv3(srcA, srcB, lh, iv, bi, Hl, Wl, dst, pad0=False, TB=1):
            """srcA: pair plane for ci 0-7; srcB: for ci 8-15. pad0: w6-style."""
            Hp_, Wp_ = dims(Hl, Wl)
            CS = Hp_ * 2 * Wp_
            Hout = Hl - 2 if pad0 else Hl
            Wout = Wl - 2 if pad0 else Wl
            NT = (Hout + 5) // 6
            roff = 2 if pad0 else 1
            for g in range(NT // TB):
                rn = rhs_pool.tile([128, RHS_W], F16, tag="rh_n")
                ts = []
                for tt in range(TB):
                    t = g * TB + tt
                    r0 = 6 * t if 6 * t + 6 <= Hout else Hout - 6
                    ts.append(r0)
                    nc.sync.dma_start(
                        rn[0:64, tt * 2 * Wp_:(tt + 1) * 2 * Wp_],
                        AP(srcA, (r0 + roff) * 2 * Wp_,
                           [[CS, 8], [1, 8 * 2 * Wp_]]))
                    nc.scalar.dma_start(
                        rn[64:128, tt * 2 * Wp_:(tt + 1) * 2 * Wp_],
                        AP(srcB, (r0 + roff) * 2 * Wp_,
                           [[CS, 8], [1, 8 * 2 * Wp_]]))
                r = next_ring('o3' if pad0 else 'out', TB * 2 * Wp_)
                whs = []
                w0 = 0
                while w0 < Wout:
                    whs.append((w0, min(320, Wout - w0)))
                    w0 += 320
                coff = 2 if pad0 else 1
                for (wo0, wcnt) in whs:
                    ps_n = psp.tile([128, 512], F32, tag="ps_n")
                    ps_d = psp.tile([128, 512], F32, tag="ps_d")
                    for dx in range(3):
                        nc.tensor.matmul(
                            ps_n[0:48, 0:TB * wcnt], lh[:, 48 * dx:48 * dx + 48],
                            fap(rn, wo0 + dx + coff, [[2 * Wp_, TB], [1, wcnt]]),
                            start=(dx == 0), stop=(dx == 2))
                    for dx in range(3):
                        nc.tensor.matmul(
                            ps_d[0:48, 0:TB * wcnt], lh[:, 48 * dx:48 * dx + 48],
                            fap(rn, Wp_ + wo0 + dx + coff, [[2 * Wp_, TB], [1, wcnt]]),
                            start=(dx == 0), stop=(dx == 2))
                    for tt in range(TB):
                        epilogue(ps_n[0:48, wcnt * tt:wcnt * tt + wcnt],
                                 ps_d[0:48, wcnt * tt:wcnt * tt + wcnt], 48, wcnt,
                                 iv, bi, r,
                                 2 * Wp_ * tt + 2 + wo0, 2 * Wp_ * tt + Wp_ + 2 + wo0)
                for tt in range(TB):
                    nc.gpsimd.dma_start(
                        AP(dst, (2 + ts[tt]) * 2 * Wp_,
                           [[CS, 8], [1, 6 * 2 * Wp_]]),
                        r[0:48, tt * 2 * Wp_:(tt + 1) * 2 * Wp_])

        # ---------------- debug dumps ----------------
        def dump(pl, nm, C, Hl, Wl, nrows=40):
            if not DEBUG_DUMPS:
                return
            Hp_, Wp_ = dims(Hl, Wl)
            dbg = nc.declare_dram_parameter(f"dbg_{nm}", [C, nrows * 2 * Wp_],
                                            F16, isOutput=True)
            nc.gpsimd.dma_start(
                dbg[:],
                AP(pl, 0, [[Hp_ * 2 * Wp_, C], [1, nrows * 2 * Wp_]]))

        # ---------------- network ----------------
        dump(P0, 'P0', 1, H, W)
        dump(P1, 'P1', 8, H, W)
        conv5(P1, P['lh2'], P['iv2'], P['bi2'], H, W, P2)
        dump(P2, 'P2', 8, H, W)
        conv5(P2, P['lh3'], P['iv3'], P['bi3'], H, W, P1f)
        dump(P1f, 'P1f', 8, H, W)
        pool_pass(P1f, H, W, P1d)
        dump(P1d, 'P1d', 8, 240, 320)
        conv5(P1d, P['lh2'], P['iv2'], P['bi2'], 240, 320, P2h)
        conv5(P2h, P['lh3'], P['iv3'], P['bi3'], 240, 320, P2d)
        dump(P2d, 'P2d', 8, 240, 320)
        pool_pass(P2d, 240, 320, P2dd)
        conv5_small(P2dd, P['lh2'], P['iv2'], P['bi2'], 120, 160, P3d, TB=2)
        dump(P3d, 'P3d', 8, 120, 160)
        pool_pass(P3d, 120, 160, P4i)
        conv5_small(P4i, P['lh2'], P['iv2'], P['bi2'], 60, 80, P4c, TB=5)
        dump(P4c, 'P4c', 8, 60, 80)
        up_pass(P4c, P4f, 60, 80)
        dump(P4f, 'P4f', 8, 120, 160)
        conv3(P3d, P4f, P['lh4'], P['iv4'], P['bi4'], 120, 160, P34, TB=2)
        dump(P34, 'P34', 8, 120, 160)
        up_pass(P34, P34f, 120, 160)
        conv3(P2d, P34f, P['lh5'], P['iv5'], P['bi5'], 240, 320, P23, TB=1)
        dump(P23, 'P23', 8, 240, 320)
        up_pass(P23, P23f, 240, 320)
        dump(P23f, 'P23f', 8, 480, 640)
        conv3(P23f, P1f, P['lh6'], P['iv6'], P['bi6'], H, W, P6, pad0=True, TB=1)
        dump(P6, 'P6', 8, 480, 640)

        # ---------------- L11: w7 1x1 (slow path, f32 out) ----------------
        lh7 = P['lh7']
        for t in range(30):
            rn = rhs_pool.tile([128, RHS_W], F16, tag="rh_n")
            # 1288 contiguous from (row 16t+1, xc, col 1): c col j lands at
            # tile col 643+j, so the c segment for col 1+k is at 644+k.
            nc.sync.dma_start(
                rn[:, 0:1288],
                AP(P6, (16 * t + 1) * W2 + 1,
                   [[2 * Hp * Wp, 8], [W2, 16], [1, 1288]]))
            xo = tmp_pool.tile([16, 640], F32, tag="f_xo")
            co_ = tmp_pool.tile([16, 640], F32, tag="f_co")
            for half in range(2):
                ps_n = psp.tile([128, 512], F32, tag="ps_n")
                ps_d = psp.tile([128, 512], F32, tag="ps_d")
                nc.tensor.matmul(ps_n[0:16, 0:320], lh7[:],
                                 rn[:, 320 * half:320 * half + 320],
                                 start=True, stop=True)
                nc.tensor.matmul(ps_d[0:16, 0:320], lh7[:],
                                 rn[:, 644 + 320 * half:644 + 320 * half + 320],
                                 start=True, stop=True)
                de = tmp_pool.tile([128, 512], F32, tag="ep_de")
                nc.scalar.activation(de[0:16, 0:320], ps_d[0:16, 0:320],
                                     ACTF.Copy, bias=EPS)
                rcp = tmp_pool.tile([128, 512], F32, tag="ep_rc")
                nc.vector.reciprocal_approx_fast(rcp[0:16, 0:320], de[0:16, 0:320])
                xt = tmp_pool.tile([128, 512], F32, tag="ep_xt")
                nc.vector.tensor_mul(xt[0:16, 0:320], ps_n[0:16, 0:320],
                                     rcp[0:16, 0:320])
                nc.scalar.activation(xo[0:16, 320 * half:320 * half + 320],
                                     xt[0:16, 0:320], ACTF.Copy, bias=b7s)
                nc.scalar.activation(co_[0:16, 320 * half:320 * half + 320],
                                     ps_d[0:16, 0:320], ACTF.Copy, scale=i7s)
            nc.gpsimd.dma_start(
                AP(out_x, (16 * t) * W, [[W, 16], [1, 640]]), xo[0:16, 0:640])
            nc.gpsimd.dma_start(
                AP(out_c, (16 * t) * W, [[W, 16], [1, 640]]), co_[0:16, 0:640])

        stack.close()
    nc.finalize()
    return nc


_CACHE = {}
TRACE = False
LAST = None


def kernel(**inputs):
    import time as _t
    key = 0
    if key not in _CACHE:
        _t0 = _t.time()
        con = prep_consts(inputs)
        nc = bacc.Bacc("TRN2", target_bir_lowering=False, debug=False)
        build(nc, con)
        print(f"[kernel] build+finalize done {_t.time()-_t0:.1f}s", flush=True)
        _CACHE[key] = (nc, con)
    nc, con = _CACHE[key]

    S = np.asarray(inputs['S'], np.float32)  # [8,1,480,640]
    in_maps = []
    for b in range(B):
        m = {'S': np.ascontiguousarray(S[b, 0])}
        for k, v in con.items():
            if isinstance(v, np.ndarray):
                m[k] = v
        in_maps.append(m)
    r = run_bass_kernel_spmd(nc, in_maps, list(range(B)), trace=TRACE)
    global LAST
    LAST = r
    res = r.results
    if TRACE and r.exec_time_ns:
        print(f"HW exec time: {r.exec_time_ns} ns", flush=True)
    xout = np.stack([res[b]['out_x'] for b in range(B)])[:, None]
    cout = np.stack([res[b]['out_c'] for b in range(B)])[:, None]
    return xout, cout


# revision 15
# speedup vs baseline: 1.6572x; 1.2791x over previous
"""Trainium2 Bass kernel for nn_DNET_61881888800848 (normalized-conv U-Net).

Data-parallel over batch: 8 samples -> 8 NeuronCores, one full network per core.

v2 scheme: H-folded Toeplitz-band convolution with row-interleaved (xc|c)
plane pairs. Feature pairs live in DRAM as [C, Hp, 2, Wp] f16 -- each frame
row stores [xc row (Wp) | conf row (Wp)] contiguously, pad=2 zero ring, frame
origin (2,2). This keeps every conv rhs gather and plane write a single <=3-dim
DMA: rhs tiles are [(ci,hi) partitions, (xc|c) x Wp cols], the band matmuls
accumulate kernel-x taps in PSUM for nomin (xc seg) and denom (c seg), and the
epilogue is 2 ACT ops + 1 DVE op (xc = isw*nomin + (b*isw)*denom, which equals
(nomin/denom + b) * c exactly; c = isw*denom), writing one interleaved ring ->
one write DMA per block. Pooling gathers (xc, c) at conf-argmax and scales by
1/4 (xc_pool = xc_sel/4 identically -- no division, no x planes anywhere).
Upsampling is DVE w-broadcast + row-duplicated interleaved writes.
"""
import sys
sys.path.insert(0, '/opt/trn_rl_repo')
import numpy as np

import concourse.bacc as bacc
import concourse.tile as tile
import concourse.mybir as mybir
from concourse.ap import AP
from concourse.bass_utils import run_bass_kernel_spmd

F32 = mybir.dt.float32
F16 = mybir.dt.float16
U8 = mybir.dt.uint8
ALU = mybir.AluOpType
ACTF = mybir.ActivationFunctionType
EPS = 1e-20

B, H, W = 8, 480, 640
PAD = 2
RHS_W = 1344  # fixed rhs tile width


def dims(h, w):
    return h + 2 * PAD, w + 2 * PAD


# ---------------- host-side weight prep ----------------

def band_lhsT(w, HI, HO, dx):
    """w: [co, ci, kh, kw] -> [(ci,HI), (co,HO)] band for kernel-x tap dx."""
    co_n, ci_n, kh, kw = w.shape
    out = np.zeros((ci_n * HI, co_n * HO), np.float32)
    for co in range(co_n):
        for ho in range(HO):
            for ci in range(ci_n):
                for ky in range(kh):
                    out[ci * HI + ho + ky, co * HO + ho] = w[co, ci, ky, dx]
    return out


def prep_consts(inputs):
    w1, w2, w3 = inputs['w1'], inputs['w2'], inputs['w3']
    w4, w5, w6, w7 = inputs['w4'], inputs['w5'], inputs['w6'], inputs['w7']
    c = {}
    # L1: K=(dx5,hi20)=100, M=(co8,ho16)=128, single matmul (dx folded into K)
    lh1 = np.zeros((100, 128), np.float32)
    for dx in range(5):
        for co in range(8):
            for ho in range(16):
                for ky in range(5):
                    lh1[dx * 20 + ho + ky, co * 16 + ho] = w1[co, 0, ky, dx]
    c['lh1'] = lh1
    # 5x5 8ch: [(c,16),(co,12)] x 5 dx
    c['lh2'] = np.stack([band_lhsT(w2, 16, 12, dx) for dx in range(5)])
    c['lh3'] = np.stack([band_lhsT(w3, 16, 12, dx) for dx in range(5)])
    # 3x3 16ch: [(c16,8),(co8,6)] x 3 dx
    c['lh4'] = np.stack([band_lhsT(w4, 8, 6, dx) for dx in range(3)])
    c['lh5'] = np.stack([band_lhsT(w5, 8, 6, dx) for dx in range(3)])
    c['lh6'] = np.stack([band_lhsT(w6, 8, 6, dx) for dx in range(3)])
    # w7 1x1: [(c8,hi16),(1,ho16)] diag
    lh7 = np.zeros((128, 16), np.float32)
    for ci in range(8):
        for ho in range(16):
            lh7[ci * 16 + ho, ho] = w7[0, ci, 0, 0]
    c['lh7'] = lh7

    def vecs(w_, b_, M_per_co):
        isw = (1.0 / w_.reshape(w_.shape[0], -1).sum(1)).astype(np.float32)
        bw = (np.asarray(b_, np.float32) * isw).astype(np.float32)
        return (np.repeat(isw, M_per_co), np.repeat(bw, M_per_co))

    c['iv1'], c['bi1'] = vecs(w1, inputs['b1'], 16)
    c['iv2'], c['bi2'] = vecs(w2, inputs['b2'], 12)
    c['iv3'], c['bi3'] = vecs(w3, inputs['b3'], 12)
    c['iv4'], c['bi4'] = vecs(w4, inputs['b4'], 6)
    c['iv5'], c['bi5'] = vecs(w5, inputs['b5'], 6)
    c['iv6'], c['bi6'] = vecs(w6, inputs['b6'], 6)
    for k in list(c.keys()):
        if k.startswith('lh'):
            c[k] = c[k].astype(np.float16)
    c['b7s'] = float(inputs['b7'][0])
    c['i7s'] = float(1.0 / w7.sum())
    return c


def fap(t, col_off, free_dims, p0=0, pn=None):
    """AP into SBUF tile t: partitions [p0:pn), explicit free dims at col_off."""
    a = t[:] if pn is None else t[p0:pn]
    return AP(a.tensor, a.offset + col_off, [list(a.ap[0])] + free_dims)


# ---------------- device program ----------------

DEBUG_DUMPS = False


def build(nc, con):
    S_in = nc.declare_dram_parameter("S", [H, W], F32, isOutput=False)
    pin = {}
    for k, v in con.items():
        if isinstance(v, np.ndarray):
            dt_ = F16 if k.startswith('lh') else F32
            pin[k] = nc.declare_dram_parameter(k, list(v.shape), dt_, isOutput=False)
    out_x = nc.declare_dram_parameter("out_x", [H, W], F32, isOutput=True)
    out_c = nc.declare_dram_parameter("out_c", [H, W], F32, isOutput=True)

    b7s, i7s = con['b7s'], con['i7s']
    Hp, Wp = dims(H, W)
    W2 = 2 * Wp  # full-res interleaved row pitch

    with tile.TileContext(nc) as tc:
        import contextlib
        stack = contextlib.ExitStack()
        sb = stack.enter_context(tc.tile_pool(name="sb", bufs=1))
        psp = stack.enter_context(tc.tile_pool(name="ps", bufs=4, space="PSUM"))
        tmp_pool = stack.enter_context(tc.tile_pool(name="tmp", bufs=4))
        rhs_pool = stack.enter_context(tc.tile_pool(name="rhs", bufs=8))

        # ---- constants in SBUF
        P = {}
        def load_const(name):
            ar = con[name]
            if ar.ndim == 3:
                nd, K, M = ar.shape
                t = sb.tile([K, nd * M], F16, tag=name)
                nc.sync.dma_start(t[:], AP(pin[name], 0,
                                           [[M, K], [K * M, nd], [1, M]]))
            else:
                K, M = ar.shape
                t = sb.tile([K, M], F16, tag=name)
                nc.sync.dma_start(t[:], pin[name][:])
            return t
        for nm in ('lh1', 'lh2', 'lh3', 'lh4', 'lh5', 'lh6', 'lh7'):
            P[nm] = load_const(nm)
        for nm in ('iv1', 'bi1', 'iv2', 'bi2', 'iv3', 'bi3', 'iv4', 'bi4',
                   'iv5', 'bi5', 'iv6', 'bi6'):
            n = con[nm].shape[0]
            t = sb.tile([n, 1], F32, tag=nm)
            nc.sync.dma_start(t[:], pin[nm][:].unsqueeze(1))
            P[nm] = t
        zt = sb.tile([128, 2576], F16, tag="zero")
        nc.vector.memset(zt[:], 0.0)

        # ---- plane pairs: [C, Hp, 2, Wp] f16 (each row: [xc | conf])
        def plane(name, C, Hl, Wl):
            Hp_, Wp_ = dims(Hl, Wl)
            return nc.dram_tensor(name, [C, Hp_, 2 * Wp_], F16)

        def zero_strips(pl, C, Hl, Wl, extra_bottom=0):
            Hp_, Wp_ = dims(Hl, Wl)
            CS = Hp_ * 2 * Wp_
            n = PAD * 2 * Wp_
            offs = [0, (Hp_ - PAD) * 2 * Wp_]
            if extra_bottom:
                offs.append((Hp_ - PAD - extra_bottom) * 2 * Wp_)
            for off in offs:
                nc.gpsimd.dma_start(
                    AP(pl, off, [[CS, C], [1, n]]), zt[0:C, 0:n])

        # P0 gets one spare row: L1's dx-shifted full-width gather reads a few
        # elements past the last frame row.
        P0 = nc.dram_tensor("P0", [1, (Hp + 1) * 2 * Wp], F16)
        P1 = plane("P1", 8, H, W)
        P2 = plane("P2", 8, H, W)
        P1f = plane("P1f", 8, H, W)
        P1d = plane("P1d", 8, 240, 320)
        P2h = plane("P2h", 8, 240, 320)
        P2d = plane("P2d", 8, 240, 320)
        P2dd = plane("P2dd", 8, 120, 160)
        P3d = plane("P3d", 8, 120, 160)
        P4i = plane("P4i", 8, 60, 80)
        P4c = plane("P4c", 8, 60, 80)
        P4f = plane("P4f", 8, 120, 160)
        P34 = plane("P34", 8, 120, 160)
        P34f = plane("P34f", 8, 240, 320)
        P23 = plane("P23", 8, 240, 320)
        P23f = plane("P23f", 8, H, W)
        P6 = plane("P6", 8, H, W)

        for (pl, C, Hl, Wl) in (
            (P0, 1, H, W), (P1, 8, H, W), (P2, 8, H, W), (P1f, 8, H, W),
            (P1d, 8, 240, 320), (P2h, 8, 240, 320), (P2d, 8, 240, 320),
            (P2dd, 8, 120, 160), (P3d, 8, 120, 160),
            (P4i, 8, 60, 80), (P4c, 8, 60, 80),
            (P4f, 8, 120, 160), (P34, 8, 120, 160),
            (P34f, 8, 240, 320), (P23, 8, 240, 320), (P23f, 8, H, W),
        ):
            zero_strips(pl, C, Hl, Wl)
        zero_strips(P6, 8, H, W, extra_bottom=2)

        # ring tiles with zero borders: memset once at creation; epilogues
        # write only interior cols of each segment, so full-width DMA writes
        # carry the plane's w-pads.
        RB = 4
        rings = {}
        rctr = [0]
        def next_ring(name, width):
            key = (name, width)
            if key not in rings:
                lst = []
                for i in range(RB):
                    t = sb.tile([128, width], F16, tag=f"r_{name}_{width}_{i}")
                    nc.vector.memset(t[:], 0.0)
                    lst.append(t)
                rings[key] = lst
            rctr[0] += 1
            return rings[key][rctr[0] % RB]

        # ---------------- generic conv epilogue ----------------
        # xc = isw*nomin + (b*isw)*denom  (== (nomin/denom + b) * c exactly)
        # c  = isw*denom
        def epilogue(ps_n, ps_d, M, NWD, ivec, bivec, ring, xc_col, c_col):
            nc.scalar.activation(ring[0:M, c_col:c_col + NWD], ps_d,
                                 ACTF.Copy, scale=ivec[0:M, :])
            t2 = tmp_pool.tile([128, 512], F32, tag="ep_t2")
            nc.scalar.activation(t2[0:M, 0:NWD], ps_d, ACTF.Copy,
                                 scale=bivec[0:M, :])
            nc.vector.scalar_tensor_tensor(ring[0:M, xc_col:xc_col + NWD],
                                           ps_n, ivec[0:M, :], t2[0:M, 0:NWD],
                                           ALU.mult, ALU.add)

        # ---------------- L0: prep (S*c0 | c0) ----------------
        for hb in range(4):
            st = tmp_pool.tile([120, W], F32, tag="prep_s")
            nc.sync.dma_start(st[:], AP(S_in, 120 * hb * W, [[W, 120], [1, W]]))
            r = next_ring('out', 1288)
            nc.vector.tensor_scalar(r[0:120, 644 + 2:644 + 2 + W], st[:], 0.01,
                                    None, ALU.is_gt)
            nc.vector.scalar_tensor_tensor(r[0:120, 2:2 + W], st[:], 0.01,
                                           st[:], ALU.is_gt, ALU.mult)
            nc.gpsimd.dma_start(
                AP(P0, (2 + 120 * hb) * W2, [[W2, 120], [1, 1288]]),
                r[0:120, 0:1288])

        # ---------------- L1 ----------------
        lh1 = P['lh1']
        for t in range(30):
            rt = rhs_pool.tile([128, RHS_W], F16, tag="rh_n")
            # partitions (dx5, hi20); 1288 contiguous = (xc 644 | c 644), the
            # dx column shift applies consistently to both segments
            nc.sync.dma_start(
                rt[0:100, 0:1288],
                AP(P0, (16 * t) * W2, [[1, 5], [W2, 20], [1, 1288]]))
            r = next_ring('out', 1288)
            for half in range(2):
                ps_n = psp.tile([128, 512], F32, tag="ps_n")
                ps_d = psp.tile([128, 512], F32, tag="ps_d")
                nc.tensor.matmul(ps_n[0:128, 0:320], lh1[0:100, :],
                                 rt[0:100, 320 * half:320 * half + 320],
                                 start=True, stop=True)
                nc.tensor.matmul(ps_d[0:128, 0:320], lh1[0:100, :],
                                 rt[0:100, 644 + 320 * half:644 + 320 * half + 320],
                                 start=True, stop=True)
                epilogue(ps_n[0:128, 0:320], ps_d[0:128, 0:320], 128, 320,
                         P['iv1'], P['bi1'], r, 2 + 320 * half, 644 + 2 + 320 * half)
            nc.gpsimd.dma_start(
                AP(P1, (2 + 16 * t) * W2, [[2 * Hp * Wp, 8], [1, 16 * 1288]]),
                r[:, 0:1288])

        # ---------------- 5x5 8ch conv layer (W >= 320) ----------------
        def conv5(src, lh, iv, bi, Hl, Wl, dst):
            Hp_, Wp_ = dims(Hl, Wl)
            CS = Hp_ * 2 * Wp_
            NT = Hl // 12
            nhalf = Wl // 320
            for t in range(NT):
                rn = rhs_pool.tile([128, RHS_W], F16, tag="rh_n")
                nc.sync.dma_start(
                    rn[:, 0:2 * Wp_],
                    AP(src, 12 * t * 2 * Wp_,
                       [[CS, 8], [2 * Wp_, 16], [1, 2 * Wp_]]))
                r = next_ring('out', 2 * Wp_)
                for half in range(nhalf):
                    ps_n = psp.tile([128, 512], F32, tag="ps_n")
                    ps_d = psp.tile([128, 512], F32, tag="ps_d")
                    for dx in range(5):
                        nc.tensor.matmul(
                            ps_n[0:96, 0:320], lh[:, 96 * dx:96 * dx + 96],
                            rn[:, 320 * half + dx:320 * half + dx + 320],
                            start=(dx == 0), stop=(dx == 4))
                    for dx in range(5):
                        nc.tensor.matmul(
                            ps_d[0:96, 0:320], lh[:, 96 * dx:96 * dx + 96],
                            rn[:, Wp_ + 320 * half + dx:Wp_ + 320 * half + dx + 320],
                            start=(dx == 0), stop=(dx == 4))
                    epilogue(ps_n[0:96, 0:320], ps_d[0:96, 0:320], 96, 320,
                             iv, bi, r, 2 + 320 * half, Wp_ + 2 + 320 * half)
                nc.gpsimd.dma_start(
                    AP(dst, (2 + 12 * t) * 2 * Wp_,
                       [[CS, 8], [1, 12 * 2 * Wp_]]),
                    r[0:96, 0:2 * Wp_])

        # small-W 5x5 conv (Wl in {160, 80}): batch TB t-blocks into free dim
        def conv5_small(src, lh, iv, bi, Hl, Wl, dst, TB):
            Hp_, Wp_ = dims(Hl, Wl)
            CS = Hp_ * 2 * Wp_
            NT = Hl // 12
            for g in range(NT // TB):
                rn = rhs_pool.tile([128, RHS_W], F16, tag="rh_n")
                for tt in range(TB):
                    t = g * TB + tt
                    nc.sync.dma_start(
                        rn[:, tt * 2 * Wp_:(tt + 1) * 2 * Wp_],
                        AP(src, 12 * t * 2 * Wp_,
                           [[CS, 8], [2 * Wp_, 16], [1, 2 * Wp_]]))
                r = next_ring('out', TB * 2 * Wp_)
                ps_n = psp.tile([128, 512], F32, tag="ps_n")
                ps_d = psp.tile([128, 512], F32, tag="ps_d")
                NWD = TB * Wl
                for dx in range(5):
                    nc.tensor.matmul(
                        ps_n[0:96, 0:NWD], lh[:, 96 * dx:96 * dx + 96],
                        fap(rn, dx, [[2 * Wp_, TB], [1, Wl]]),
                        start=(dx == 0), stop=(dx == 4))
                for dx in range(5):
                    nc.tensor.matmul(
                        ps_d[0:96, 0:NWD], lh[:, 96 * dx:96 * dx + 96],
                        fap(rn, Wp_ + dx, [[2 * Wp_, TB], [1, Wl]]),
                        start=(dx == 0), stop=(dx == 4))
                for tt in range(TB):
                    epilogue(ps_n[0:96, Wl * tt:Wl * tt + Wl],
                             ps_d[0:96, Wl * tt:Wl * tt + Wl], 96, Wl,
                             iv, bi, r, 2 * Wp_ * tt + 2, 2 * Wp_ * tt + Wp_ + 2)
                for tt in range(TB):
                    t = g * TB + tt
                    nc.gpsimd.dma_start(
                        AP(dst, (2 + 12 * t) * 2 * Wp_,
                           [[CS, 8], [1, 12 * 2 * Wp_]]),
                        r[0:96, tt * 2 * Wp_:(tt + 1) * 2 * Wp_])

        # ---------------- pool pass: 2x2 first-max on conf ----------------
        # xc_out = xc[argmax]/4, c_out = c[argmax]/4 (no division needed)
        def pool_pass(src, Hl, Wl, dst):
            Hp_, Wp_ = dims(Hl, Wl)
            CS = Hp_ * 2 * Wp_
            Ho, Wo = Hl // 2, Wl // 2
            Hpo, Wpo = dims(Ho, Wo)
            CSo = Hpo * 2 * Wpo
            blocks = []
            h0 = 0
            while h0 < Ho:
                blocks.append(min(h0, Ho - 16))
                h0 += 16
            for hb in sorted(set(blocks)):
                T = {}
                for (nm, soff) in (('x', 0), ('c', Wp_)):
                    for dy in range(2):
                        tt = tmp_pool.tile([128, 640], F16, tag=f"po_{nm}{dy}")
                        nc.sync.dma_start(
                            tt[:, 0:Wl],
                            AP(src, (2 * hb + dy + 2) * 2 * Wp_ + soff + 2,
                               [[CS, 8], [4 * Wp_, 16], [1, Wl]]))
                        T[(nm, dy)] = tt
                cw = {}
                xw = {}
                for dy in range(2):
                    m = tmp_pool.tile([128, 324], U8, tag=f"po_m{dy}")
                    cwt = tmp_pool.tile([128, 324], F16, tag=f"po_cw{dy}")
                    xwt = tmp_pool.tile([128, 324], F16, tag=f"po_xw{dy}")
                    ca = T[('c', dy)][:, 0:Wl:2]
                    cb = T[('c', dy)][:, 1:Wl:2]
                    nc.vector.tensor_tensor(m[:, 0:Wo], ca, cb, ALU.is_ge)
                    nc.vector.tensor_tensor(cwt[:, 0:Wo], ca, cb, ALU.max)
                    nc.scalar.activation(xwt[:, 0:Wo], T[('x', dy)][:, 1:Wl:2],
                                         ACTF.Copy)
                    nc.vector.copy_predicated(xwt[:, 0:Wo], m[:, 0:Wo],
                                              T[('x', dy)][:, 0:Wl:2])
                    cw[dy] = cwt
                    xw[dy] = xwt
                m3 = tmp_pool.tile([128, 324], U8, tag="po_m3")
                nc.vector.tensor_tensor(m3[:, 0:Wo], cw[0][:, 0:Wo],
                                        cw[1][:, 0:Wo], ALU.is_ge)
                r = next_ring('po', 2 * Wpo)
                nc.vector.tensor_tensor(r[:, Wpo + 2:Wpo + 2 + Wo],
                                        cw[0][:, 0:Wo], cw[1][:, 0:Wo], ALU.max)
                nc.vector.tensor_scalar(r[:, Wpo + 2:Wpo + 2 + Wo],
                                        r[:, Wpo + 2:Wpo + 2 + Wo], 0.25,
                                        None, ALU.mult)
                nc.scalar.activation(r[:, 2:2 + Wo], xw[1][:, 0:Wo], ACTF.Copy)
                nc.vector.copy_predicated(r[:, 2:2 + Wo], m3[:, 0:Wo],
                                          xw[0][:, 0:Wo])
                nc.vector.tensor_scalar(r[:, 2:2 + Wo], r[:, 2:2 + Wo], 0.25,
                                        None, ALU.mult)
                nc.gpsimd.dma_start(
                    AP(dst, (2 + hb) * 2 * Wpo, [[CSo, 8], [1, 16 * 2 * Wpo]]),
                    r[:, 0:2 * Wpo])

        # ---------------- upsample pass (both planes of a pair) ----------------
        def up_pass(src, dst, Hc, Wc):
            Hpc, Wpc = dims(Hc, Wc)
            Hpf, Wpf = dims(2 * Hc, 2 * Wc)
            CSc = Hpc * 2 * Wpc
            CSf = Hpf * 2 * Wpf
            blocks = []
            h0 = 0
            while h0 < Hc:
                blocks.append(min(h0, Hc - 16))
                h0 += 16
            for hb in sorted(set(blocks)):
                ct = tmp_pool.tile([128, 648], F16, tag="up_c")
                nc.sync.dma_start(
                    ct[:, 0:2 * Wpc],
                    AP(src, (hb + 2) * 2 * Wpc,
                       [[2 * Wpc, 16], [CSc, 8], [1, 2 * Wpc]]))
                wex = next_ring('up', 2 * Wpf)
                # width-double both segments: out[seg][2w+e] = in[seg][w]
                nc.vector.tensor_copy(
                    fap(wex, 2, [[Wpf, 2], [1, 2 * Wc]]),
                    fap(ct, 2, [[Wpc, 2], [1, Wc], [0, 2]]))
                for dy in range(2):
                    nc.gpsimd.dma_start(
                        AP(dst, (2 * hb + dy + 2) * 2 * Wpf,
                           [[4 * Wpf, 16], [CSf, 8], [1, 2 * Wpf]]),
                        wex[:, 0:2 * Wpf])

        # ---------------- 3x3 16ch conv layer ----------------
        def conv3(srcA, srcB, lh, iv, bi, Hl, Wl, dst, pad0=False, TB=1):
            """srcA: pair plane for ci 0-7; srcB: for ci 8-15. pad0: w6-style."""
            Hp_, Wp_ = dims(Hl, Wl)
            CS = Hp_ * 2 * Wp_
            Hout = Hl - 2 if pad0 else Hl
            Wout = Wl - 2 if pad0 else Wl
            NT = (Hout + 5) // 6
            roff = 2 if pad0 else 1
            for g in range(NT // TB):
                rn = rhs_pool.tile([128, RHS_W], F16, tag="rh_n")
                ts = []
                for tt in range(TB):
                    t = g * TB + tt
                    r0 = 6 * t if 6 * t + 6 <= Hout else Hout - 6
                    ts.append(r0)
                    nc.sync.dma_start(
                        rn[0:64, tt * 2 * Wp_:(tt + 1) * 2 * Wp_],
                        AP(srcA, (r0 + roff) * 2 * Wp_,
                           [[CS, 8], [1, 8 * 2 * Wp_]]))
                    nc.sync.dma_start(
                        rn[64:128, tt * 2 * Wp_:(tt + 1) * 2 * Wp_],
                        AP(srcB, (r0 + roff) * 2 * Wp_,
                           [[CS, 8], [1, 8 * 2 * Wp_]]))
                r = next_ring('o3' if pad0 else 'out', TB * 2 * Wp_)
                whs = []
                w0 = 0
                while w0 < Wout:
                    whs.append((w0, min(320, Wout - w0)))
                    w0 += 320
                coff = 2 if pad0 else 1
                for (wo0, wcnt) in whs:
                    ps_n = psp.tile([128, 512], F32, tag="ps_n")
                    ps_d = psp.tile([128, 512], F32, tag="ps_d")
                    for dx in range(3):
                        nc.tensor.matmul(
                            ps_n[0:48, 0:TB * wcnt], lh[:, 48 * dx:48 * dx + 48],
                            fap(rn, wo0 + dx + coff, [[2 * Wp_, TB], [1, wcnt]]),
                            start=(dx == 0), stop=(dx == 2))
                    for dx in range(3):
                        nc.tensor.matmul(
                            ps_d[0:48, 0:TB * wcnt], lh[:, 48 * dx:48 * dx + 48],
                            fap(rn, Wp_ + wo0 + dx + coff, [[2 * Wp_, TB], [1, wcnt]]),
                            start=(dx == 0), stop=(dx == 2))
                    for tt in range(TB):
                        epilogue(ps_n[0:48, wcnt * tt:wcnt * tt + wcnt],
                                 ps_d[0:48, wcnt * tt:wcnt * tt + wcnt], 48, wcnt,
                                 iv, bi, r,
                                 2 * Wp_ * tt + 2 + wo0, 2 * Wp_ * tt + Wp_ + 2 + wo0)
                for tt in range(TB):
                    nc.gpsimd.dma_start(
                        AP(dst, (2 + ts[tt]) * 2 * Wp_,
                           [[CS, 8], [1, 6 * 2 * Wp_]]),
                        r[0:48, tt * 2 * Wp_:(tt + 1) * 2 * Wp_])

        # ---------------- debug dumps ----------------
        def dump(pl, nm, C, Hl, Wl, nrows=40):
            if not DEBUG_DUMPS:
                return
            Hp_, Wp_ = dims(Hl, Wl)
            dbg = nc.declare_dram_parameter(f"dbg_{nm}", [C, nrows * 2 * Wp_],
                                            F16, isOutput=True)
            nc.gpsimd.dma_start(
                dbg[:],
                AP(pl, 0, [[Hp_ * 2 * Wp_, C], [1, nrows * 2 * Wp_]]))

        # ---------------- network ----------------
        dump(P0, 'P0', 1, H, W)
        dump(P1, 'P1', 8, H, W)
        conv5(P1, P['lh2'], P['iv2'], P['bi2'], H, W, P2)
        dump(P2, 'P2', 8, H, W)
        conv5(P2, P['lh3'], P['iv3'], P['bi3'], H, W, P1f)
        dump(P1f, 'P1f', 8, H, W)
        pool_pass(P1f, H, W, P1d)
        dump(P1d, 'P1d', 8, 240, 320)
        conv5(P1d, P['lh2'], P['iv2'], P['bi2'], 240, 320, P2h)
        conv5(P2h, P['lh3'], P['iv3'], P['bi3'], 240, 320, P2d)
        dump(P2d, 'P2d', 8, 240, 320)
        pool_pass(P2d, 240, 320, P2dd)
        conv5_small(P2dd, P['lh2'], P['iv2'], P['bi2'], 120, 160, P3d, TB=2)
        dump(P3d, 'P3d', 8, 120, 160)
        pool_pass(P3d, 120, 160, P4i)
        conv5_small(P4i, P['lh2'], P['iv2'], P['bi2'], 60, 80, P4c, TB=5)
        dump(P4c, 'P4c', 8, 60, 80)
        up_pass(P4c, P4f, 60, 80)
        dump(P4f, 'P4f', 8, 120, 160)
        conv3(P3d, P4f, P['lh4'], P['iv4'], P['bi4'], 120, 160, P34, TB=2)
        dump(P34, 'P34', 8, 120, 160)
        up_pass(P34, P34f, 120, 160)
        conv3(P2d, P34f, P['lh5'], P['iv5'], P['bi5'], 240, 320, P23, TB=1)
        dump(P23, 'P23', 8, 240, 320)
        up_pass(P23, P23f, 240, 320)
        dump(P23f, 'P23f', 8, 480, 640)
        conv3(P23f, P1f, P['lh6'], P['iv6'], P['bi6'], H, W, P6, pad0=True, TB=1)
        dump(P6, 'P6', 8, 480, 640)

        # ---------------- L11: w7 1x1 (slow path, f32 out) ----------------
        lh7 = P['lh7']
        for t in range(30):
            rn = rhs_pool.tile([128, RHS_W], F16, tag="rh_n")
            # 1288 contiguous from (row 16t+1, xc, col 1): c col j lands at
            # tile col 643+j, so the c segment for col 1+k is at 644+k.
            nc.sync.dma_start(
                rn[:, 0:1288],
                AP(P6, (16 * t + 1) * W2 + 1,
                   [[2 * Hp * Wp, 8], [W2, 16], [1, 1288]]))
            xo = tmp_pool.tile([16, 640], F32, tag="f_xo")
            co_ = tmp_pool.tile([16, 640], F32, tag="f_co")
            for half in range(2):
                ps_n = psp.tile([128, 512], F32, tag="ps_n")
                ps_d = psp.tile([128, 512], F32, tag="ps_d")
                nc.tensor.matmul(ps_n[0:16, 0:320], lh7[:],
                                 rn[:, 320 * half:320 * half + 320],
                                 start=True, stop=True)
                nc.tensor.matmul(ps_d[0:16, 0:320], lh7[:],
                                 rn[:, 644 + 320 * half:644 + 320 * half + 320],
                                 start=True, stop=True)
                de = tmp_pool.tile([128, 512], F32, tag="ep_de")
                nc.scalar.activation(de[0:16, 0:320], ps_d[0:16, 0:320],
                                     ACTF.Copy, bias=EPS)
                rcp = tmp_pool.tile([128, 512], F32, tag="ep_rc")
                nc.vector.reciprocal_approx_fast(rcp[0:16, 0:320], de[0:16, 0:320])
                xt = tmp_pool.tile([128, 512], F32, tag="ep_xt")
                nc.vector.tensor_mul(xt[0:16, 0:320], ps_n[0:16, 0:320],
                                     rcp[0:16, 0:320])
                nc.scalar.activation(xo[0:16, 320 * half:320 * half + 320],
                                     xt[0:16, 0:320], ACTF.Copy, bias=b7s)
                nc.scalar.activation(co_[0:16, 320 * half:320 * half + 320],
                                     ps_d[0:16, 0:320], ACTF.Copy, scale=i7s)
            nc.gpsimd.dma_start(
                AP(out_x, (16 * t) * W, [[W, 16], [1, 640]]), xo[0:16, 0:640])
            nc.gpsimd.dma_start(
                AP(out_c, (16 * t) * W, [[W, 16], [1, 640]]), co_[0:16, 0:640])

        stack.close()
    nc.finalize()
    return nc


_CACHE = {}
TRACE = False
LAST = None


def kernel(**inputs):
    import time as _t
    key = 0
    if key not in _CACHE:
        _t0 = _t.time()
        con = prep_consts(inputs)
        nc = bacc.Bacc("TRN2", target_bir_lowering=False, debug=False)
        build(nc, con)
        print(f"[kernel] build+finalize done {_t.time()-_t0:.1f}s", flush=True)
        _CACHE[key] = (nc, con)
    nc, con = _CACHE[key]

    S = np.asarray(inputs['S'], np.float32)  # [8,1,480,640]
    in_maps = []
    for b in range(B):
        m = {'S': np.ascontiguousarray(S[b, 0])}
        for k, v in con.items():
            if isinstance(v, np.ndarray):
                m[k] = v
        in_maps.append(m)
    r = run_bass_kernel_spmd(nc, in_maps, list(range(B)), trace=TRACE)
    global LAST
    LAST = r
    res = r.results
    if TRACE and r.exec_time_ns:
        print(f"HW exec time: {r.exec_time_ns} ns", flush=True)
    xout = np.stack([res[b]['out_x'] for b in range(B)])[:, None]
    cout = np.stack([res[b]['out_c'] for b in range(B)])[:, None]
    return xout, cout
